# revision 1
# baseline (speedup 1.0000x reference)
"""Trainium2 Bass kernel for non-causal multi-head self-attention (B=2, T=2048,
C=1024, H=16, hd=64), SPMD over 8 NeuronCores.

Sharding: 2-way data parallel on batch x 4-way sequence parallel on query
tokens. Core c handles batch c//4, query tokens [512*(c%4), 512*(c%4+1)).
Collectives on this fleet have a ~100us fixed cost, so instead of
AllGathering k/v each core redundantly computes full k and v for its batch.
All 16 heads of attention run locally for the core's 512 queries; the output
projection is fully local, so the output needs no collective (the host
concatenates token slices).

Structure / tricks:
- Host marshals x.T / W_attn.T / W_proj.T pre-transposed, pre-cast to bf16.
- v is stored with a ones-column per head; the PV matmul (lhsT = v_aug)
  then yields softmax denominators as row 64 of y for free.
- No max-subtraction in softmax (logits are ~N(0,1); exp is safe in fp32).
- Head-pair row-tiling: two K=64 S.T-matmuls run concurrently in PE row
  groups (0,0)/(64,0), writing one [128,1024] PSUM tile that is exp'd by a
  single ScalarE activation (amortizes the per-op overhead).
- k is computed lazily: head-pair unit u>=2 is produced during attention
  pair u-2, filling the PE slack of the ScalarE-bound softmax phase and
  keeping the HAM clock-gate at 2.4GHz.
- v-bias folded exactly into an adjusted b_proj on host (softmax rows sum
  to 1); q/k biases applied as free per-partition bias in epilogues.
- 1/denominator broadcast across partitions via a K=1 PE outer product,
  emitted one head-pair late so the DVE reciprocal is off the PE critical
  path; the final pair's normalize hides under partial proj accumulation.
"""

import sys

for _p in ("/opt/trn_rl_repo",):
    if _p not in sys.path:
        sys.path.insert(0, _p)

import numpy as np
import ml_dtypes

import concourse.bass as bass
import concourse.mybir as mybir
import concourse.tile as tile
from concourse import bacc
from concourse.bass_utils import run_bass_kernel_spmd

BF16 = mybir.dt.bfloat16
F32 = mybir.dt.float32
AF = mybir.ActivationFunctionType

B, T, C = 2, 2048, 1024
H, HD = 16, 64
N_CORES = 8
G = 4              # sequence-parallel degree
TQ = T // G        # local query tokens per core (512)
PAIRS = H // 2     # head pairs (8)
KT = T // 128      # key tiles (16)
CT = C // 128      # contraction tiles over C (8)
VW = HD + 1        # v columns per head incl. ones column (65)
SCALE = 1.0 / np.sqrt(HD)

_CACHE = {}


def build_nc():
    nc = bacc.Bacc(None, target_bir_lowering=False, debug=False, num_devices=N_CORES)

    xT = nc.declare_dram_parameter("xT", [C, T], BF16, isOutput=False)
    wT = nc.declare_dram_parameter("wT", [C, 3 * C], BF16, isOutput=False)
    wpT = nc.declare_dram_parameter("wpT", [C, C], BF16, isOutput=False)
    bqk = nc.declare_dram_parameter("bqk", [128, 16], F32, isOutput=False)
    bp = nc.declare_dram_parameter("bp", [128, 8], F32, isOutput=False)
    xq = nc.declare_dram_parameter("xq", [C, TQ], BF16, isOutput=False)
    out = nc.declare_dram_parameter("out", [C, TQ], F32, isOutput=True)

    with tile.TileContext(nc) as tc:
        with tc.tile_pool(name="sb", bufs=1) as sb:
            # ---- persistent SBUF (live through attention) ----
            q_sb = [sb.tile([128, TQ], BF16, tag=f"q{p}", name=f"q{p}") for p in range(PAIRS)]
            k_sb = [sb.tile([128, T], BF16, tag=f"k{p}", name=f"k{p}") for p in range(PAIRS)]
            v_sb = [sb.tile([128, H * VW], BF16, tag=f"v{t}", name=f"v{t}") for t in range(KT)]
            yn_sb = [sb.tile([128, TQ], BF16, tag=f"yn{p}", name=f"yn{p}") for p in range(PAIRS)]
            wpt = [sb.tile([128, C], BF16, tag=f"wpt{p}", name=f"wpt{p}") for p in range(PAIRS)]
            bqk_sb = sb.tile([128, 16], F32, tag="bqk", name="bqk")
            bp_sb = sb.tile([128, 8], F32, tag="bp", name="bp")
            ones_sb = sb.tile([1, HD], F32, tag="ones", name="ones")

            nc.sync.dma_start(out=bqk_sb[:, :], in_=bqk[:, :])
            nc.sync.dma_start(out=bp_sb[:, :], in_=bp[:, :])
            nc.vector.memset(ones_sb[:, :], 1.0)

            # ones columns of v tiles (set once, v epilogues write around them)
            for t in range(KT):
                vh = v_sb[t][:, :].rearrange("p (h c) -> p h c", c=VW)
                nc.vector.memset(vh[:, :, HD:HD + 1], 1.0)

            # k-weights and full x.T stay resident through attention for the
            # lazily-computed k units
            sb_kx = tc.alloc_tile_pool(name="sb_kx", bufs=1)
            xt = [sb_kx.tile([128, T], BF16, tag=f"xt{k}", name=f"xt{k}") for k in range(CT)]
            wtk = [sb_kx.tile([128, C], BF16, tag=f"wtk{k}", name=f"wtk{k}") for k in range(CT)]

            def k_lazy_gen(pool, tag="kacc", bufs=2):
                """Generator emitting the k-units 2..7 one matmul per next();
                epilogues (DVE) at chunk boundaries. Yields the unit id that
                is fully emitted so far (or the last one when done)."""
                for u in range(2, PAIRS):
                    acc = None
                    for j in range(4 * CT):
                        ch, k = divmod(j, CT)
                        if k == 0:
                            acc = pool.tile([128, TQ], F32, tag=tag, name=tag, bufs=bufs)
                        nc.tensor.matmul(
                            acc[:, :],
                            lhsT=wtk[k][:, 128 * u:128 * (u + 1)],
                            rhs=xt[k][:, 512 * ch:512 * (ch + 1)],
                            start=(k == 0), stop=(k == CT - 1),
                        )
                        if k == CT - 1:
                            nc.vector.tensor_scalar_add(
                                k_sb[u][:, 512 * ch:512 * (ch + 1)],
                                acc[:, :], bqk_sb[:, 8 + u:9 + u],
                            )
                        yield u if j == 4 * CT - 1 else u - 1

            def k_unit(u, ch, pool, epilogue_engine, tag="kacc", bufs=2):
                acc = pool.tile([128, TQ], F32, tag=tag, name=tag, bufs=bufs)
                for k in range(CT):
                    nc.tensor.matmul(
                        acc[:, :],
                        lhsT=wtk[k][:, 128 * u:128 * (u + 1)],
                        rhs=xt[k][:, 512 * ch:512 * (ch + 1)],
                        start=(k == 0), stop=(k == CT - 1),
                    )
                dst = k_sb[u][:, 512 * ch:512 * (ch + 1)]
                if epilogue_engine == "act":
                    nc.scalar.activation(
                        dst, acc[:, :], AF.Identity, bias=bqk_sb[:, 8 + u:9 + u],
                    )
                else:
                    nc.vector.tensor_scalar_add(dst, acc[:, :], bqk_sb[:, 8 + u:9 + u])

            # ---- phase 1: q, k-units 0..1, full v ----
            with tc.tile_pool(name="sb_qv", bufs=1) as sb_qv, \
                 tc.tile_pool(name="ps_qkv", bufs=1, space="PSUM") as ps1:
                xqt = [sb_qv.tile([128, TQ], BF16, tag=f"xqt{k}", name=f"xqt{k}") for k in range(CT)]
                wtq = [sb_qv.tile([128, C], BF16, tag=f"wtq{k}", name=f"wtq{k}") for k in range(CT)]
                wtv = [sb_qv.tile([128, C], BF16, tag=f"wtv{k}", name=f"wtv{k}") for k in range(CT)]
                for k in range(CT):
                    nc.sync.dma_start(out=wtq[k][:, :], in_=wT[128 * k:128 * (k + 1), 0:C])
                    nc.sync.dma_start(out=xqt[k][:, :], in_=xq[128 * k:128 * (k + 1), :])
                for k in range(CT):
                    nc.sync.dma_start(out=xt[k][:, :], in_=xT[128 * k:128 * (k + 1), :])
                    nc.sync.dma_start(out=wtk[k][:, :], in_=wT[128 * k:128 * (k + 1), C:2 * C])
                    nc.sync.dma_start(out=wtv[k][:, :], in_=wT[128 * k:128 * (k + 1), 2 * C:3 * C])
                for p in range(PAIRS):
                    nc.sync.dma_start(out=wpt[p][:, :], in_=wpT[128 * p:128 * (p + 1), :])

                # q (needs only 2.6MB of DMA -> starts earliest)
                for m in range(8):
                    acc = ps1.tile([128, TQ], F32, tag="qk", name="qk", bufs=3)
                    for k in range(CT):
                        nc.tensor.matmul(
                            acc[:, :],
                            lhsT=wtq[k][:, 128 * m:128 * (m + 1)],
                            rhs=xqt[k][:, :],
                            start=(k == 0), stop=(k == CT - 1),
                        )
                    nc.scalar.activation(
                        q_sb[m][:, :], acc[:, :],
                        AF.Identity, bias=bqk_sb[:, m:m + 1],
                    )

                # k units 0..1 (pairs 0 and 1); units 2..7 computed lazily
                for u in range(2):
                    for ch in range(4):
                        k_unit(u, ch, ps1, "act", tag="qk", bufs=3)

                # v for all T tokens, token-major, strided into per-head
                # 65-column slots (ones columns already set)
                for t in range(KT):
                    vacc = ps1.tile([128, C], F32, tag="v", name="v", bufs=2)
                    for k in range(CT):
                        for h2 in range(2):
                            nc.tensor.matmul(
                                vacc[:, 512 * h2:512 * (h2 + 1)],
                                lhsT=xt[k][:, 128 * t:128 * (t + 1)],
                                rhs=wtv[k][:, 512 * h2:512 * (h2 + 1)],
                                start=(k == 0), stop=(k == CT - 1),
                            )
                    nc.vector.tensor_copy(
                        v_sb[t][:, :].rearrange("p (h c) -> p h c", c=VW)[:, :, 0:HD],
                        vacc[:, :].rearrange("p (h c) -> p h c", c=HD),
                    )

            # ---- phase 2: attention per head-pair ----
            deferred = [None]
            sbatt = tc.alloc_tile_pool(name="sbatt", bufs=1)
            with tc.tile_pool(name="ps_att", bufs=1, space="PSUM") as ps2:

                def emit_normalize(item, pool, bc_tag, bc_bufs):
                    p, ystA, ystB = item
                    for half, yst in ((0, ystA), (1, ystB)):
                        rc = sbatt.tile([1, TQ], F32, tag="recip", name="recip", bufs=2)
                        nc.vector.reciprocal(rc[:, :], yst[HD:HD + 1, :])
                        bc = pool.tile([HD, TQ], F32, tag=bc_tag, name=bc_tag, bufs=bc_bufs)
                        nc.tensor.matmul(
                            bc[:, :], lhsT=ones_sb[:, :], rhs=rc[:, :],
                            start=True, stop=True,
                        )
                        nc.vector.tensor_mul(
                            yn_sb[p][64 * half:64 * (half + 1), :],
                            yst[0:HD, :], bc[:, :],
                        )

                kgen = k_lazy_gen(ps2)
                kdone = [1]

                def kstep(n):
                    for _ in range(n):
                        kdone[0] = next(kgen, PAIRS)

                for p in range(PAIRS):
                    # safety: unit p must be fully emitted before pair p reads it
                    while kdone[0] < p:
                        kstep(1)
                    ya = ps2.tile([VW, TQ], F32, tag="yA", name="yA")
                    yb = ps2.tile([VW, TQ], F32, tag="yB", name="yB")
                    for t in range(KT):
                        sp = ps2.tile([128, 2 * TQ], F32, tag="sp", name="sp", bufs=2)
                        nc.tensor.matmul(
                            sp[:, 0:TQ],
                            lhsT=k_sb[p][0:64, 128 * t:128 * (t + 1)],
                            rhs=q_sb[p][0:64, :],
                            start=True, stop=True,
                        )
                        nc.tensor.matmul(
                            sp[:, TQ:2 * TQ],
                            lhsT=k_sb[p][64:128, 128 * t:128 * (t + 1)],
                            rhs=q_sb[p][64:128, :],
                            start=True, stop=True,
                            tile_position=(64, 0),
                        )
                        pab = sbatt.tile([128, 2 * TQ], BF16, tag="pab", name="pab", bufs=4)
                        nc.scalar.activation(
                            pab[:, :], sp[:, :], AF.Exp, scale=float(SCALE),
                        )
                        nc.tensor.matmul(
                            ya[:, :],
                            lhsT=v_sb[t][:, VW * 2 * p:VW * 2 * p + VW],
                            rhs=pab[:, 0:TQ],
                            start=(t == 0), stop=(t == KT - 1),
                        )
                        nc.tensor.matmul(
                            yb[:, :],
                            lhsT=v_sb[t][:, VW * (2 * p + 1):VW * (2 * p + 1) + VW],
                            rhs=pab[:, TQ:2 * TQ],
                            start=(t == 0), stop=(t == KT - 1),
                        )
                        kstep(2 if t < 12 else 1)
                        if t == 4 and deferred[0] is not None:
                            emit_normalize(deferred[0], ps2, "sp", 2)
                            deferred[0] = None
                    # free y PSUM banks right away (ScalarE copy; its next exp
                    # is gated on the next pair's S anyway)
                    ystA = sbatt.tile([VW, TQ], F32, tag="ystA", name="ystA", bufs=2)
                    ystB = sbatt.tile([VW, TQ], F32, tag="ystB", name="ystB", bufs=2)
                    nc.vector.tensor_copy(ystA[:, :], ya[:, :])
                    nc.vector.tensor_copy(ystB[:, :], yb[:, :])
                    deferred[0] = (p, ystA, ystB)

            # ---- phase 3: proj halves d=0..3, p=0..6 partial, then the
            # last pair's normalize hides under the partial accumulation
            if True:
                with tc.tile_pool(name="ps_proj", bufs=1, space="PSUM") as ps3:
                    pacc = [ps3.tile([128, TQ], F32, tag=f"proj{d}", name=f"proj{d}")
                            for d in range(4)]
                    for d in range(4):
                        for p in range(PAIRS - 1):
                            nc.tensor.matmul(
                                pacc[d][:, :],
                                lhsT=wpt[p][:, 128 * d:128 * (d + 1)],
                                rhs=yn_sb[p][:, :],
                                start=(p == 0), stop=False,
                            )
                    emit_normalize(deferred[0], ps3, "bc", 2)
                    for d in range(4):
                        nc.tensor.matmul(
                            pacc[d][:, :],
                            lhsT=wpt[PAIRS - 1][:, 128 * d:128 * (d + 1)],
                            rhs=yn_sb[PAIRS - 1][:, :],
                            start=False, stop=True,
                        )
                        otmp = sbatt.tile([128, TQ], F32, tag="otmp", name="otmp", bufs=4)
                        nc.scalar.activation(
                            otmp[:, :], pacc[d][:, :], AF.Identity,
                            bias=bp_sb[:, d:d + 1],
                        )
                        nc.sync.dma_start(
                            out=out[128 * d:128 * (d + 1), :], in_=otmp[:, :]
                        )
                    for d in range(4, 8):
                        acc = ps3.tile([128, TQ], F32, tag=f"proj{d-4}", name="projb", bufs=1)
                        for p in range(PAIRS):
                            nc.tensor.matmul(
                                acc[:, :],
                                lhsT=wpt[p][:, 128 * d:128 * (d + 1)],
                                rhs=yn_sb[p][:, :],
                                start=(p == 0), stop=(p == PAIRS - 1),
                            )
                        otmp = sbatt.tile([128, TQ], F32, tag="otmp", name="otmp", bufs=4)
                        nc.scalar.activation(
                            otmp[:, :], acc[:, :], AF.Identity,
                            bias=bp_sb[:, d:d + 1],
                        )
                        nc.sync.dma_start(
                            out=out[128 * d:128 * (d + 1), :], in_=otmp[:, :]
                        )

            sbatt.release()
            sb_kx.release()

    nc.compile()
    return nc


def _get_nc():
    if "nc" not in _CACHE:
        _CACHE["nc"] = build_nc()
    return _CACHE["nc"]


def make_in_maps(x, W_attn, b_attn, W_proj, b_proj):
    x = np.asarray(x, dtype=np.float32)
    W_attn = np.asarray(W_attn, dtype=np.float32)
    b_attn = np.asarray(b_attn, dtype=np.float32)
    W_proj = np.asarray(W_proj, dtype=np.float32)
    b_proj = np.asarray(b_proj, dtype=np.float32)

    bf = ml_dtypes.bfloat16
    wT = np.ascontiguousarray(W_attn.T).astype(bf)          # [C, 3C]
    wpT = np.ascontiguousarray(W_proj.T).astype(bf)         # [C, C]
    bqk = np.ascontiguousarray(b_attn[:2 * C].reshape(16, 128).T)  # [128, 16]
    b_v = b_attn[2 * C:]
    bp_adj = b_proj + W_proj @ b_v                           # fold v-bias exactly
    bp = np.ascontiguousarray(bp_adj.reshape(8, 128).T)      # [128, 8]

    xTg = [np.ascontiguousarray(x[g].T).astype(bf) for g in range(B)]  # [C, T]

    in_maps = []
    for c in range(N_CORES):
        g, r = divmod(c, G)
        in_maps.append({
            "xT": xTg[g],
            "xq": np.ascontiguousarray(xTg[g][:, TQ * r:TQ * (r + 1)]),
            "wT": wT, "wpT": wpT, "bqk": bqk, "bp": bp,
        })
    return in_maps


def run_shards(in_maps, trace=False, **kw):
    nc = _get_nc()
    return run_bass_kernel_spmd(
        nc, in_maps, core_ids=list(range(N_CORES)), trace=trace, **kw
    )


def kernel(x, W_attn, b_attn, W_proj, b_proj):
    in_maps = make_in_maps(x, W_attn, b_attn, W_proj, b_proj)
    res = run_shards(in_maps)
    out = np.empty((B, T, C), dtype=np.float32)
    for c in range(N_CORES):
        g, r = divmod(c, G)
        out[g, TQ * r:TQ * (r + 1), :] = res.results[c]["out"].T
    return out



# revision 6
# speedup vs baseline: 1.0818x; 1.0818x over previous
"""Trainium2 Bass kernel for non-causal multi-head self-attention (B=2, T=2048,
C=1024, H=16, hd=64), SPMD over 8 NeuronCores.

Sharding: 2-way data parallel on batch x 4-way tensor parallel on heads.
Core c handles batch c//4 and heads [4*(c%4), 4*(c%4)+4) (= 2 head pairs)
for ALL 2048 tokens. c_attn is column-split, c_proj is row-split: each core
emits a PARTIAL output projection [C, T]; the host unshard sums the 4
partials per batch and adds the (host-folded) bias. No device collectives.
This removes the 4x-redundant k/v projections of a seq-parallel split
(~86us of PE time per core).

Structure / tricks (inherited from the seq-parallel baseline):
- Host marshals x.T / W-slices pre-transposed, pre-cast to bf16.
- v stored with a ones-column per head; PV matmul yields softmax
  denominators as row 64 of y for free.
- No max-subtraction in softmax (logits ~N(0,1); exp safe in fp32).
- Head-pair row-tiling: two K=64 S.T-matmuls run concurrently in PE row
  groups (0,0)/(64,0) writing one [128,1024] PSUM tile exp'd by a single
  ScalarE activation.
- Lazy qkv: only q(pair0,chunk0)+k(pair0)+v(pair0) are computed eagerly;
  the rest streams into PE slack during the ScalarE-bound attention.
- PV accumulates both heads of a pair into one [65,1024] PSUM tile; one
  DVE copy + one approx-reciprocal + one PE ones-broadcast + two DVE mults
  normalize the pair-chunk into yn [128(pair y-dims), T].
- proj contracts K=128 per pair (single matmul per (pair, outblock));
  proj of query-chunk c is emitted as soon as both pairs' yn(c) exist,
  hiding proj + output DMA under the tail of attention.
- q/k biases via free per-partition DVE tensor_scalar_add epilogues;
  v-bias and b_proj folded exactly into a host-side bias (softmax rows
  sum to 1), added after the host reduction.
"""

import sys

for _p in ("/opt/trn_rl_repo",):
    if _p not in sys.path:
        sys.path.insert(0, _p)

import numpy as np
import ml_dtypes

import concourse.bass as bass
import concourse.mybir as mybir
import concourse.tile as tile
from concourse import bacc
from concourse.bass_utils import run_bass_kernel_spmd

BF16 = mybir.dt.bfloat16
F32 = mybir.dt.float32
AF = mybir.ActivationFunctionType

B, T, C = 2, 2048, 1024
H, HD = 16, 64
N_CORES = 8
G = 4               # head-quad parallel degree (within a batch)
HC = H // G         # heads per core (4)
PAIRS = HC // 2     # head pairs per core (2)
QC = 4              # query chunks
TQ = T // QC        # queries per chunk (512)
KT = T // 128       # key tiles (16)
CT = C // 128       # contraction tiles over C (8)
VW = HD + 1         # v columns per head incl. ones column (65)
SCALE = 1.0 / np.sqrt(HD)

_CACHE = {}


def build_nc():
    nc = bacc.Bacc(None, target_bir_lowering=False, debug=False, num_devices=N_CORES)

    xT = nc.declare_dram_parameter("xT", [C, T], BF16, isOutput=False)
    wT = nc.declare_dram_parameter("wT", [C, 768], BF16, isOutput=False)
    wpT = nc.declare_dram_parameter("wpT", [256, C], BF16, isOutput=False)
    bqk = nc.declare_dram_parameter("bqk", [128, 4], F32, isOutput=False)
    out = nc.declare_dram_parameter("out", [C, T], F32, isOutput=True)

    with tile.TileContext(nc) as tc:
        with tc.tile_pool(name="sb", bufs=1) as sb:
            # ---- persistent SBUF ----
            q_sb = [sb.tile([128, T], BF16, tag=f"q{p}", name=f"q{p}") for p in range(PAIRS)]
            k_sb = [sb.tile([128, T], BF16, tag=f"k{p}", name=f"k{p}") for p in range(PAIRS)]
            v_sb = [sb.tile([128, HC * VW], BF16, tag=f"v{t}", name=f"v{t}") for t in range(KT)]
            yn_sb = [sb.tile([128, T], BF16, tag=f"yn{p}", name=f"yn{p}") for p in range(PAIRS)]
            wpt = [sb.tile([128, C], BF16, tag=f"wpt{p}", name=f"wpt{p}") for p in range(PAIRS)]
            bqk_sb = sb.tile([128, 4], F32, tag="bqk", name="bqk")
            ones_sb = sb.tile([1, HD], F32, tag="ones", name="ones")

            xt = [sb.tile([128, T], BF16, tag=f"xt{k}", name=f"xt{k}") for k in range(CT)]
            wtq = [sb.tile([128, 256], BF16, tag=f"wtq{k}", name=f"wtq{k}") for k in range(CT)]
            wtk = [sb.tile([128, 256], BF16, tag=f"wtk{k}", name=f"wtk{k}") for k in range(CT)]
            wtv = [sb.tile([128, 256], BF16, tag=f"wtv{k}", name=f"wtv{k}") for k in range(CT)]

            nc.sync.dma_start(out=bqk_sb[:, :], in_=bqk[:, :])
            nc.vector.memset(ones_sb[:, :], 1.0)
            for t in range(KT):
                vh = v_sb[t][:, :].rearrange("p (h c) -> p h c", c=VW)
                nc.vector.memset(vh[:, :, HD:HD + 1], 1.0)

            # weights first (small), then x
            for k in range(CT):
                nc.sync.dma_start(out=wtk[k][:, :], in_=wT[128 * k:128 * (k + 1), 256:512])
                nc.sync.dma_start(out=wtv[k][:, :], in_=wT[128 * k:128 * (k + 1), 512:768])
                nc.sync.dma_start(out=wtq[k][:, :], in_=wT[128 * k:128 * (k + 1), 0:256])
            for p in range(PAIRS):
                nc.sync.dma_start(out=wpt[p][:, :], in_=wpT[128 * p:128 * (p + 1), :])
            for k in range(CT):
                nc.sync.dma_start(out=xt[k][:, :], in_=xT[128 * k:128 * (k + 1), :])

            with tc.tile_pool(name="ps", bufs=1, space="PSUM") as ps:

                def emit_q(p, ch):
                    acc = ps.tile([128, TQ], F32, tag="lz", name="qacc", bufs=2)
                    for k in range(CT):
                        nc.tensor.matmul(
                            acc[:, :],
                            lhsT=wtq[k][:, 128 * p:128 * (p + 1)],
                            rhs=xt[k][:, TQ * ch:TQ * (ch + 1)],
                            start=(k == 0), stop=(k == CT - 1),
                        )
                        yield None
                    nc.vector.tensor_scalar_add(
                        q_sb[p][:, TQ * ch:TQ * (ch + 1)], acc[:, :],
                        bqk_sb[:, p:p + 1],
                    )

                def emit_k(p, ch):
                    acc = ps.tile([128, TQ], F32, tag="lz", name="kacc", bufs=2)
                    for k in range(CT):
                        nc.tensor.matmul(
                            acc[:, :],
                            lhsT=wtk[k][:, 128 * p:128 * (p + 1)],
                            rhs=xt[k][:, TQ * ch:TQ * (ch + 1)],
                            start=(k == 0), stop=(k == CT - 1),
                        )
                        yield None
                    nc.vector.tensor_scalar_add(
                        k_sb[p][:, TQ * ch:TQ * (ch + 1)], acc[:, :],
                        bqk_sb[:, 2 + p:3 + p],
                    )

                def emit_v(p, t):
                    acc = ps.tile([128, 128], F32, tag="lz", name="vacc", bufs=2)
                    for k in range(CT):
                        nc.tensor.matmul(
                            acc[:, :],
                            lhsT=xt[k][:, 128 * t:128 * (t + 1)],
                            rhs=wtv[k][:, 128 * p:128 * (p + 1)],
                            start=(k == 0), stop=(k == CT - 1),
                        )
                        yield None
                    vh = v_sb[t][:, :].rearrange("p (h c) -> p h c", c=VW)
                    nc.vector.tensor_copy(
                        vh[:, 2 * p:2 * p + 2, 0:HD],
                        acc[:, :].rearrange("p (h c) -> p h c", c=HD),
                    )

                # ---- phase A: eager k(0), v(0), q(0,0) ----
                for gen in (
                    [emit_k(0, ch) for ch in range(QC)]
                    + [emit_v(0, t) for t in range(KT)]
                    + [emit_q(0, 0)]
                ):
                    for _ in gen:
                        pass

                # ---- lazy stream for the rest of qkv ----
                done = set()

                def lazy_stream():
                    for ch in range(1, QC):
                        yield from emit_q(0, ch)
                        done.add(("q", 0, ch))
                    for ch in range(QC):
                        yield from emit_k(1, ch)
                    done.add(("k", 1))
                    for ch in range(QC):
                        yield from emit_q(1, ch)
                        done.add(("q", 1, ch))
                    for t in range(KT):
                        yield from emit_v(1, t)
                    done.add(("v", 1))

                lz = lazy_stream()

                def pump(n):
                    for _ in range(n):
                        next(lz, None)

                def require(key):
                    while key not in done:
                        if next(lz, "END") == "END":
                            assert key in done, f"lazy stream ended before {key}"

                # ---- attention + deferred normalize + streaming proj ----
                sb_att = tc.alloc_tile_pool(name="sb_att", bufs=1)
                deferred = [None]
                proj_ready = []
                proj_queue = []

                def emit_normalize(item):
                    p, ch, yst = item
                    rc = sb_att.tile([1, 2 * TQ], F32, tag="recip", name="recip", bufs=2)
                    nc.vector.reciprocal(rc[:, :], yst[HD:HD + 1, :])
                    bc = ps.tile([HD, 2 * TQ], F32, tag="sp", name="bc", bufs=2)
                    # one matmul per PSUM bank (N<=512 fp32)
                    nc.tensor.matmul(
                        bc[:, 0:TQ], lhsT=ones_sb[:, :], rhs=rc[:, 0:TQ],
                        start=True, stop=True,
                    )
                    nc.tensor.matmul(
                        bc[:, TQ:2 * TQ], lhsT=ones_sb[:, :], rhs=rc[:, TQ:2 * TQ],
                        start=True, stop=True,
                    )
                    nc.vector.tensor_mul(
                        yn_sb[p][0:HD, TQ * ch:TQ * (ch + 1)],
                        yst[0:HD, 0:TQ], bc[:, 0:TQ],
                    )
                    nc.vector.tensor_mul(
                        yn_sb[p][HD:128, TQ * ch:TQ * (ch + 1)],
                        yst[0:HD, TQ:2 * TQ], bc[:, TQ:2 * TQ],
                    )
                    if p == PAIRS - 1:
                        proj_ready.append(ch)

                def emit_proj_chunk(ch):
                    for d in range(CT):
                        pacc = ps.tile([128, TQ], F32, tag="lz", name="pacc", bufs=2)
                        for p in range(PAIRS):
                            nc.tensor.matmul(
                                pacc[:, :],
                                lhsT=wpt[p][:, 128 * d:128 * (d + 1)],
                                rhs=yn_sb[p][:, TQ * ch:TQ * (ch + 1)],
                                start=(p == 0), stop=(p == PAIRS - 1),
                            )
                        otmp = sb_att.tile([128, TQ], F32, tag="otmp", name="otmp", bufs=4)
                        nc.vector.tensor_copy(otmp[:, :], pacc[:, :])
                        nc.sync.dma_start(
                            out=out[128 * d:128 * (d + 1), TQ * ch:TQ * (ch + 1)],
                            in_=otmp[:, :],
                        )
                        yield None

                def pump_proj(n):
                    for _ in range(n):
                        if proj_queue:
                            if next(proj_queue[0], "END") == "END":
                                proj_queue.pop(0)

                for p in range(PAIRS):
                    for ch in range(QC):
                        if not (p == 0 and ch == 0):
                            require(("q", p, ch))
                        if p > 0:
                            require(("k", p))
                            require(("v", p))
                        yab = ps.tile([VW, 2 * TQ], F32, tag="yab", name="yab")
                        for t in range(KT):
                            sp = ps.tile([128, 2 * TQ], F32, tag="sp", name="sp", bufs=2)
                            nc.tensor.matmul(
                                sp[:, 0:TQ],
                                lhsT=k_sb[p][0:64, 128 * t:128 * (t + 1)],
                                rhs=q_sb[p][0:64, TQ * ch:TQ * (ch + 1)],
                                start=True, stop=True,
                            )
                            nc.tensor.matmul(
                                sp[:, TQ:2 * TQ],
                                lhsT=k_sb[p][64:128, 128 * t:128 * (t + 1)],
                                rhs=q_sb[p][64:128, TQ * ch:TQ * (ch + 1)],
                                start=True, stop=True,
                                tile_position=(64, 0),
                            )
                            pab = sb_att.tile([128, 2 * TQ], BF16, tag="pab", name="pab", bufs=4)
                            nc.scalar.activation(
                                pab[:, :], sp[:, :], AF.Exp, scale=float(SCALE),
                            )
                            vh = v_sb[t][:, :].rearrange("p (h c) -> p h c", c=VW)
                            nc.tensor.matmul(
                                yab[:, 0:TQ],
                                lhsT=vh[:, 2 * p, :],
                                rhs=pab[:, 0:TQ],
                                start=(t == 0), stop=(t == KT - 1),
                            )
                            nc.tensor.matmul(
                                yab[:, TQ:2 * TQ],
                                lhsT=vh[:, 2 * p + 1, :],
                                rhs=pab[:, TQ:2 * TQ],
                                start=(t == 0), stop=(t == KT - 1),
                            )
                            pump(2)
                            pump_proj(1)
                            if t == 4 and deferred[0] is not None:
                                emit_normalize(deferred[0])
                                deferred[0] = None
                                while proj_ready:
                                    proj_queue.append(emit_proj_chunk(proj_ready.pop(0)))
                        yst = sb_att.tile([VW, 2 * TQ], F32, tag="yst", name="yst", bufs=2)
                        nc.vector.tensor_copy(yst[:, :], yab[:, :])
                        deferred[0] = (p, ch, yst)

                # ---- tail: last normalize + remaining proj ----
                pump(10 ** 9)
                emit_normalize(deferred[0])
                while proj_ready:
                    proj_queue.append(emit_proj_chunk(proj_ready.pop(0)))
                pump_proj(10 ** 9)

                sb_att.release()

    nc.compile()
    return nc


def _get_nc():
    if "nc" not in _CACHE:
        _CACHE["nc"] = build_nc()
    return _CACHE["nc"]


def make_in_maps(x, W_attn, b_attn, W_proj, b_proj):
    x = np.asarray(x, dtype=np.float32)
    W_attn = np.asarray(W_attn, dtype=np.float32)
    b_attn = np.asarray(b_attn, dtype=np.float32)
    W_proj = np.asarray(W_proj, dtype=np.float32)

    bf = ml_dtypes.bfloat16
    xTg = [np.ascontiguousarray(x[g].T).astype(bf) for g in range(B)]  # [C, T]

    in_maps = []
    for c in range(N_CORES):
        g, hq = divmod(c, G)
        r0 = hq * 256  # first row of this core's q slice within W_attn[0:C]
        wslice = np.concatenate(
            [W_attn[r0:r0 + 256],                   # q rows
             W_attn[C + r0:C + r0 + 256],           # k rows
             W_attn[2 * C + r0:2 * C + r0 + 256]],  # v rows
            axis=0,
        )  # [768, C]
        wT = np.ascontiguousarray(wslice.T).astype(bf)  # [C, 768]
        wpT = np.ascontiguousarray(W_proj[:, r0:r0 + 256].T).astype(bf)  # [256, C]
        bq = b_attn[r0:r0 + 256].reshape(2, 128).T
        bk = b_attn[C + r0:C + r0 + 256].reshape(2, 128).T
        bqk_h = np.ascontiguousarray(
            np.concatenate([bq, bk], axis=1), dtype=np.float32
        )
        in_maps.append({
            "xT": xTg[g], "wT": wT, "wpT": wpT, "bqk": bqk_h,
        })
    return in_maps


def run_shards(in_maps, trace=False, **kw):
    nc = _get_nc()
    return run_bass_kernel_spmd(
        nc, in_maps, core_ids=list(range(N_CORES)), trace=trace, **kw
    )


def combine_outputs(res, W_proj, b_proj, b_attn):
    """Sum the 4 head-quad partial outputs per batch, add folded bias."""
    W_proj = np.asarray(W_proj, dtype=np.float32)
    b_proj = np.asarray(b_proj, dtype=np.float32)
    b_attn = np.asarray(b_attn, dtype=np.float32)
    b_adj = b_proj + W_proj @ b_attn[2 * C:]
    out = np.empty((B, T, C), dtype=np.float32)
    for g in range(B):
        acc = res.results[g * G + 0]["out"].astype(np.float32)
        for hq in range(1, G):
            acc = acc + res.results[g * G + hq]["out"]
        out[g] = acc.T + b_adj
    return out


def kernel(x, W_attn, b_attn, W_proj, b_proj):
    in_maps = make_in_maps(x, W_attn, b_attn, W_proj, b_proj)
    res = run_shards(in_maps)
    return combine_outputs(res, W_proj, b_proj, b_attn)


# revision 8
# speedup vs baseline: 1.3117x; 1.2125x over previous
"""Trainium2 Bass kernel for non-causal multi-head self-attention (B=2, T=2048,
C=1024, H=16, hd=64), SPMD over 8 NeuronCores.

Sharding: 2-way data parallel on batch x 4-way tensor parallel on heads.
Core c handles batch c//4 and heads [4*(c%4), 4*(c%4)+4) (= 2 head pairs)
for ALL 2048 tokens. c_attn is column-split, c_proj is row-split: each core
emits a PARTIAL output projection [C, T] in bf16; the host unshard sums the
4 partials per batch and adds the (host-folded) bias. No device collectives.
This removes the 4x-redundant k/v projections of a seq-parallel split.

Structure / tricks:
- Host marshals x.T / W-slices pre-transposed, pre-cast to bf16.
- v stored with a trailing ones-column per head; the PV matmul then
  yields softmax denominators as row 64 of y for free. The y-copy splits
  dims (rows 0:64) and denominator (row 64 -> partition 0 of a separate
  tile): vector.reciprocal_approx_fast mis-executes on partition offset
  64 (verified on HW), and engines require 32-aligned partition starts.
- No max-subtraction in softmax (logits ~N(0,1); exp is safe in fp32).
- Head-pair row-tiling: two K=64 S.T-matmuls run concurrently in PE row
  groups (0,0)/(64,0) writing one [128,1024] PSUM tile exp'd by a single
  ScalarE activation (ScalarE exp is the overall bottleneck engine).
- x.T is DMA'd in four 512-token column chunks; k/v/q of pair 0 are
  emitted per-chunk so attention starts as soon as chunk 0 lands.
- Lazy qkv: the rest of q/k/v streams into PE slack during the
  ScalarE-bound attention via a pumped generator.
- PV accumulates both heads of a pair into one [65,1024] PSUM tile; one
  DVE copy + one approx-reciprocal + one PE ones-broadcast + two DVE mults
  normalize the pair-chunk into yn [128(pair y-dims), T] bf16.
- proj contracts K=128 per pair (single matmul per (pair, outblock));
  proj of query-chunk c is emitted as soon as both pairs' yn(c) exist,
  hiding proj + output DMA under the tail of attention.
- q/k biases via per-partition DVE tensor_scalar_add epilogues; v-bias
  and b_proj folded exactly into a host-side bias (softmax rows sum to
  1), added after the host reduction.
"""

import sys

for _p in ("/opt/trn_rl_repo",):
    if _p not in sys.path:
        sys.path.insert(0, _p)

import numpy as np
import ml_dtypes

import concourse.bass as bass
import concourse.mybir as mybir
import concourse.tile as tile
from concourse import bacc
from concourse.bass_utils import run_bass_kernel_spmd

BF16 = mybir.dt.bfloat16
F32 = mybir.dt.float32
AF = mybir.ActivationFunctionType

B, T, C = 2, 2048, 1024
H, HD = 16, 64
N_CORES = 8
G = 4               # head-quad parallel degree (within a batch)
HC = H // G         # heads per core (4)
PAIRS = HC // 2     # head pairs per core (2)
QC = 4              # query chunks
TQ = T // QC        # queries per chunk (512)
KT = T // 128       # key tiles (16)
CT = C // 128       # contraction tiles over C (8)
VW = HD + 1         # v columns per head incl. leading ones column (65)
SCALE = 1.0 / np.sqrt(HD)

_CACHE = {}


def build_nc():
    nc = bacc.Bacc(None, target_bir_lowering=False, debug=False, num_devices=N_CORES)

    xT = nc.declare_dram_parameter("xT", [C, T], BF16, isOutput=False)
    wT = nc.declare_dram_parameter("wT", [C, 768], BF16, isOutput=False)
    wpT = nc.declare_dram_parameter("wpT", [256, C], BF16, isOutput=False)
    bqk = nc.declare_dram_parameter("bqk", [128, 4], F32, isOutput=False)
    out = nc.declare_dram_parameter("out", [C, T], BF16, isOutput=True)

    with tile.TileContext(nc) as tc:
        with tc.tile_pool(name="sb", bufs=1) as sb:
            # ---- persistent SBUF ----
            q_sb = [sb.tile([128, T], BF16, tag=f"q{p}", name=f"q{p}") for p in range(PAIRS)]
            k_sb = [sb.tile([128, T], BF16, tag=f"k{p}", name=f"k{p}") for p in range(PAIRS)]
            v_sb = [sb.tile([128, HC * VW], BF16, tag=f"v{t}", name=f"v{t}") for t in range(KT)]
            yn_sb = [sb.tile([128, T], BF16, tag=f"yn{p}", name=f"yn{p}") for p in range(PAIRS)]
            wpt = [sb.tile([128, C], BF16, tag=f"wpt{p}", name=f"wpt{p}") for p in range(PAIRS)]
            bqk_sb = sb.tile([128, 4], F32, tag="bqk", name="bqk")
            ones_sb = sb.tile([1, HD], F32, tag="ones", name="ones")

            # x.T resident as [CT][QC] tiles of [128, TQ] (column chunks)
            xt = [[sb.tile([128, TQ], BF16, tag=f"xt{k}_{ch}", name=f"xt{k}_{ch}")
                   for ch in range(QC)] for k in range(CT)]
            wtq = [sb.tile([128, 256], BF16, tag=f"wtq{k}", name=f"wtq{k}") for k in range(CT)]
            wtk = [sb.tile([128, 256], BF16, tag=f"wtk{k}", name=f"wtk{k}") for k in range(CT)]
            wtv = [sb.tile([128, 256], BF16, tag=f"wtv{k}", name=f"wtv{k}") for k in range(CT)]

            nc.vector.memset(ones_sb[:, :], 1.0)
            for t in range(KT):
                vh = v_sb[t][:, :].rearrange("p (h c) -> p h c", c=VW)
                nc.vector.memset(vh[:, :, HD:HD + 1], 1.0)

            # DMA order = need order: qkv weights, biases, x chunks, wp last
            for k in range(CT):
                nc.sync.dma_start(out=wtk[k][:, :], in_=wT[128 * k:128 * (k + 1), 256:512])
                nc.sync.dma_start(out=wtv[k][:, :], in_=wT[128 * k:128 * (k + 1), 512:768])
                nc.sync.dma_start(out=wtq[k][:, :], in_=wT[128 * k:128 * (k + 1), 0:256])
            nc.sync.dma_start(out=bqk_sb[:, :], in_=bqk[:, :])
            for ch in range(QC):
                for k in range(CT):
                    nc.sync.dma_start(
                        out=xt[k][ch][:, :],
                        in_=xT[128 * k:128 * (k + 1), TQ * ch:TQ * (ch + 1)],
                    )
            for p in range(PAIRS):
                nc.sync.dma_start(out=wpt[p][:, :], in_=wpT[128 * p:128 * (p + 1), :])

            with tc.tile_pool(name="ps", bufs=1, space="PSUM") as ps:

                def emit_q(p, ch):
                    acc = ps.tile([128, TQ], F32, tag="lz", name="qacc", bufs=2)
                    for k in range(CT):
                        nc.tensor.matmul(
                            acc[:, :],
                            lhsT=wtq[k][:, 128 * p:128 * (p + 1)],
                            rhs=xt[k][ch][:, :],
                            start=(k == 0), stop=(k == CT - 1),
                        )
                        yield None
                    nc.vector.tensor_scalar_add(
                        q_sb[p][:, TQ * ch:TQ * (ch + 1)], acc[:, :],
                        bqk_sb[:, p:p + 1],
                    )

                def emit_k(p, ch):
                    acc = ps.tile([128, TQ], F32, tag="lz", name="kacc", bufs=2)
                    for k in range(CT):
                        nc.tensor.matmul(
                            acc[:, :],
                            lhsT=wtk[k][:, 128 * p:128 * (p + 1)],
                            rhs=xt[k][ch][:, :],
                            start=(k == 0), stop=(k == CT - 1),
                        )
                        yield None
                    nc.vector.tensor_scalar_add(
                        k_sb[p][:, TQ * ch:TQ * (ch + 1)], acc[:, :],
                        bqk_sb[:, 2 + p:3 + p],
                    )

                def emit_v(p, t):
                    acc = ps.tile([128, 128], F32, tag="lz", name="vacc", bufs=2)
                    for k in range(CT):
                        nc.tensor.matmul(
                            acc[:, :],
                            lhsT=xt[k][t // 4][:, 128 * (t % 4):128 * (t % 4 + 1)],
                            rhs=wtv[k][:, 128 * p:128 * (p + 1)],
                            start=(k == 0), stop=(k == CT - 1),
                        )
                        yield None
                    vh = v_sb[t][:, :].rearrange("p (h c) -> p h c", c=VW)
                    nc.vector.tensor_copy(
                        vh[:, 2 * p:2 * p + 2, 0:HD],
                        acc[:, :].rearrange("p (h c) -> p h c", c=HD),
                    )

                # ---- phase A: eager pair-0 qkv, emitted per x column chunk ----
                for ch in range(QC):
                    gens = [emit_k(0, ch)]
                    if ch == 0:
                        gens.append(emit_q(0, 0))
                    gens += [emit_v(0, t) for t in range(4 * ch, 4 * ch + 4)]
                    for gen in gens:
                        for _ in gen:
                            pass

                # ---- lazy stream for the rest of qkv ----
                done = set()

                def lazy_stream():
                    for ch in range(1, QC):
                        yield from emit_q(0, ch)
                        done.add(("q", 0, ch))
                    for ch in range(QC):
                        yield from emit_k(1, ch)
                    done.add(("k", 1))
                    for ch in range(QC):
                        yield from emit_q(1, ch)
                        done.add(("q", 1, ch))
                    for t in range(KT):
                        yield from emit_v(1, t)
                    done.add(("v", 1))

                lz = lazy_stream()

                def pump(n):
                    for _ in range(n):
                        next(lz, None)

                def require(key):
                    while key not in done:
                        if next(lz, "END") == "END":
                            assert key in done, f"lazy stream ended before {key}"

                # ---- attention + deferred normalize + streaming proj ----
                sb_att = tc.alloc_tile_pool(name="sb_att", bufs=1)
                deferred = [None]
                proj_ready = []
                proj_queue = []

                def emit_normalize(item):
                    p, ch, yst, den = item
                    rc = sb_att.tile([1, 2 * TQ], F32, tag="recip", name="recip", bufs=2)
                    nc.vector.reciprocal_approx_fast(rc[:, :], den[:, :])
                    bc = ps.tile([HD, 2 * TQ], F32, tag="sp", name="bc", bufs=2)
                    # one matmul per PSUM bank (N<=512 fp32)
                    nc.tensor.matmul(
                        bc[:, 0:TQ], lhsT=ones_sb[:, :], rhs=rc[:, 0:TQ],
                        start=True, stop=True,
                    )
                    nc.tensor.matmul(
                        bc[:, TQ:2 * TQ], lhsT=ones_sb[:, :], rhs=rc[:, TQ:2 * TQ],
                        start=True, stop=True,
                    )
                    nc.vector.tensor_mul(
                        yn_sb[p][0:HD, TQ * ch:TQ * (ch + 1)],
                        yst[:, 0:TQ], bc[:, 0:TQ],
                    )
                    nc.vector.tensor_mul(
                        yn_sb[p][HD:128, TQ * ch:TQ * (ch + 1)],
                        yst[:, TQ:2 * TQ], bc[:, TQ:2 * TQ],
                    )
                    if p == PAIRS - 1:
                        proj_ready.append(ch)

                def emit_proj_chunk(ch):
                    for d in range(CT):
                        pacc = ps.tile([128, TQ], F32, tag="lz", name="pacc", bufs=2)
                        for p in range(PAIRS):
                            nc.tensor.matmul(
                                pacc[:, :],
                                lhsT=wpt[p][:, 128 * d:128 * (d + 1)],
                                rhs=yn_sb[p][:, TQ * ch:TQ * (ch + 1)],
                                start=(p == 0), stop=(p == PAIRS - 1),
                            )
                        otmp = sb_att.tile([128, TQ], BF16, tag="otmp", name="otmp", bufs=4)
                        nc.vector.tensor_copy(otmp[:, :], pacc[:, :])
                        nc.sync.dma_start(
                            out=out[128 * d:128 * (d + 1), TQ * ch:TQ * (ch + 1)],
                            in_=otmp[:, :],
                        )
                        yield None

                def pump_proj(n):
                    for _ in range(n):
                        if proj_queue:
                            if next(proj_queue[0], "END") == "END":
                                proj_queue.pop(0)

                for p in range(PAIRS):
                    for ch in range(QC):
                        if not (p == 0 and ch == 0):
                            require(("q", p, ch))
                        if p > 0:
                            require(("k", p))
                            require(("v", p))
                        yab = ps.tile([VW, 2 * TQ], F32, tag="yab", name="yab")
                        for t in range(KT):
                            sp = ps.tile([128, 2 * TQ], F32, tag="sp", name="sp", bufs=2)
                            nc.tensor.matmul(
                                sp[:, 0:TQ],
                                lhsT=k_sb[p][0:64, 128 * t:128 * (t + 1)],
                                rhs=q_sb[p][0:64, TQ * ch:TQ * (ch + 1)],
                                start=True, stop=True,
                            )
                            nc.tensor.matmul(
                                sp[:, TQ:2 * TQ],
                                lhsT=k_sb[p][64:128, 128 * t:128 * (t + 1)],
                                rhs=q_sb[p][64:128, TQ * ch:TQ * (ch + 1)],
                                start=True, stop=True,
                                tile_position=(64, 0),
                            )
                            pab = sb_att.tile([128, 2 * TQ], BF16, tag="pab", name="pab", bufs=4)
                            nc.scalar.activation(
                                pab[:, :], sp[:, :], AF.Exp, scale=float(SCALE),
                            )
                            vh = v_sb[t][:, :].rearrange("p (h c) -> p h c", c=VW)
                            nc.tensor.matmul(
                                yab[:, 0:TQ],
                                lhsT=vh[:, 2 * p, :],
                                rhs=pab[:, 0:TQ],
                                start=(t == 0), stop=(t == KT - 1),
                            )
                            nc.tensor.matmul(
                                yab[:, TQ:2 * TQ],
                                lhsT=vh[:, 2 * p + 1, :],
                                rhs=pab[:, TQ:2 * TQ],
                                start=(t == 0), stop=(t == KT - 1),
                            )
                            pump(2)
                            pump_proj(1)
                            if t == 4 and deferred[0] is not None:
                                emit_normalize(deferred[0])
                                deferred[0] = None
                                while proj_ready:
                                    proj_queue.append(emit_proj_chunk(proj_ready.pop(0)))
                        yst = sb_att.tile([HD, 2 * TQ], F32, tag="yst", name="yst", bufs=2)
                        den = sb_att.tile([1, 2 * TQ], F32, tag="den", name="den", bufs=2)
                        nc.vector.tensor_copy(yst[:, :], yab[0:HD, :])
                        nc.vector.tensor_copy(den[0:1, :], yab[HD:VW, :])
                        deferred[0] = (p, ch, yst, den)

                # ---- tail: last normalize + remaining proj ----
                pump(10 ** 9)
                emit_normalize(deferred[0])
                while proj_ready:
                    proj_queue.append(emit_proj_chunk(proj_ready.pop(0)))
                pump_proj(10 ** 9)

                sb_att.release()

    nc.compile()
    return nc


def _get_nc():
    if "nc" not in _CACHE:
        _CACHE["nc"] = build_nc()
    return _CACHE["nc"]


def make_in_maps(x, W_attn, b_attn, W_proj, b_proj):
    x = np.asarray(x, dtype=np.float32)
    W_attn = np.asarray(W_attn, dtype=np.float32)
    b_attn = np.asarray(b_attn, dtype=np.float32)
    W_proj = np.asarray(W_proj, dtype=np.float32)

    bf = ml_dtypes.bfloat16
    xTg = [np.ascontiguousarray(x[g].T).astype(bf) for g in range(B)]  # [C, T]

    in_maps = []
    for c in range(N_CORES):
        g, hq = divmod(c, G)
        r0 = hq * 256  # first row of this core's q slice within W_attn[0:C]
        wslice = np.concatenate(
            [W_attn[r0:r0 + 256],                   # q rows
             W_attn[C + r0:C + r0 + 256],           # k rows
             W_attn[2 * C + r0:2 * C + r0 + 256]],  # v rows
            axis=0,
        )  # [768, C]
        wT = np.ascontiguousarray(wslice.T).astype(bf)  # [C, 768]
        wpT = np.ascontiguousarray(W_proj[:, r0:r0 + 256].T).astype(bf)  # [256, C]
        bq = b_attn[r0:r0 + 256].reshape(2, 128).T
        bk = b_attn[C + r0:C + r0 + 256].reshape(2, 128).T
        bqk_h = np.ascontiguousarray(
            np.concatenate([bq, bk], axis=1), dtype=np.float32
        )
        in_maps.append({
            "xT": xTg[g], "wT": wT, "wpT": wpT, "bqk": bqk_h,
        })
    return in_maps


def run_shards(in_maps, trace=False, **kw):
    nc = _get_nc()
    return run_bass_kernel_spmd(
        nc, in_maps, core_ids=list(range(N_CORES)), trace=trace, **kw
    )


def combine_outputs(res, W_proj, b_proj, b_attn):
    """Sum the 4 head-quad partial outputs per batch, add folded bias."""
    W_proj = np.asarray(W_proj, dtype=np.float32)
    b_proj = np.asarray(b_proj, dtype=np.float32)
    b_attn = np.asarray(b_attn, dtype=np.float32)
    b_adj = b_proj + W_proj @ b_attn[2 * C:]
    out = np.empty((B, T, C), dtype=np.float32)
    for g in range(B):
        acc = res.results[g * G + 0]["out"].astype(np.float32)
        for hq in range(1, G):
            acc = acc + res.results[g * G + hq]["out"].astype(np.float32)
        out[g] = acc.T + b_adj
    return out


def kernel(x, W_attn, b_attn, W_proj, b_proj):
    in_maps = make_in_maps(x, W_attn, b_attn, W_proj, b_proj)
    res = run_shards(in_maps)
    return combine_outputs(res, W_proj, b_proj, b_attn)


# revision 10
# speedup vs baseline: 1.3815x; 1.0532x over previous
"""Trainium2 Bass kernel for non-causal multi-head self-attention (B=2, T=2048,
C=1024, H=16, hd=64), SPMD over 8 NeuronCores.

Sharding: 2-way data parallel on batch x 4-way tensor parallel on heads.
Core c handles batch c//4 and heads [4*(c%4), 4*(c%4)+4) (= 2 head pairs)
for ALL 2048 tokens. c_attn is column-split, c_proj is row-split: each core
emits a PARTIAL output projection [C, T] in bf16; the host unshard sums the
4 partials per batch and adds the (host-folded) bias. No device collectives.
This removes the 4x-redundant k/v projections of a seq-parallel split.

Structure / tricks:
- Host marshals x.T / W-slices pre-transposed, pre-cast to bf16.
- All inputs arrive in 9 large strided DMAs (DMA issue on the Sync queue
  costs ~600ns each; 59 small DMAs serialized ~35us of startup).
- x.T lands in four 512-token column chunks; attention unit (pair0,
  chunk0) starts as soon as chunk 0 + its weights are in, with k(0,ch)/
  v(tiles) pulled on demand (require()) as later x chunks stream in.
- v for ALL 4 heads is produced by one matmul stream (N=256 instead of
  2x N=128: the PE has ~170ns fixed overhead per matmul, so small-N
  matmuls are disproportionately expensive).
- v stores a trailing ones-column per head; the PV matmul then yields
  softmax denominators as row 64 of y for free. The y-copy splits dims
  (rows 0:64) and denominator (row 64 -> partition 0 of a separate
  tile): vector.reciprocal_approx_fast mis-executes on partition offset
  64 (verified on HW), and engines require 32-aligned partition starts.
- No max-subtraction in softmax (logits ~N(0,1); exp is safe in fp32).
- Head-pair row-tiling: two K=64 S.T-matmuls run concurrently in PE row
  groups (0,0)/(64,0) writing one [128,1024] PSUM tile exp'd by a single
  ScalarE activation (ScalarE exp, 142us/core, is the bottleneck floor).
- Lazy qkv: q(0,1..3), k(1,*), q(1,*) stream into PE slack during the
  ScalarE-bound attention via a pumped generator; this also keeps the
  PE continuously busy so its DVFS state stays at the fast clock.
- PV accumulates both heads of a pair into one [65,1024] PSUM tile; one
  DVE copy + one approx-reciprocal + one PE ones-broadcast + two DVE
  mults normalize the pair-chunk into yn [128(pair y-dims), T] bf16.
  bc tiles allocate from the lazy PSUM tag, NOT the sp tag, so the
  sp double-buffer rotation (which paces S-matmul vs exp) is undisturbed.
- proj contracts K=128 per pair (single matmul per (pair, outblock));
  proj of query-chunk c is emitted as soon as both pairs' yn(c) exist
  (pumped every other key tile), hiding proj + output DMA under
  attention; only chunk 3's proj trails.
- q/k biases via per-partition DVE tensor_scalar_add epilogues; v-bias
  and b_proj folded exactly into a host-side bias (softmax rows sum to
  1), added after the host reduction.
"""

import sys

for _p in ("/opt/trn_rl_repo",):
    if _p not in sys.path:
        sys.path.insert(0, _p)

import numpy as np
import ml_dtypes

import concourse.bass as bass
import concourse.mybir as mybir
import concourse.tile as tile
from concourse import bacc
from concourse.bass_utils import run_bass_kernel_spmd

BF16 = mybir.dt.bfloat16
F32 = mybir.dt.float32
AF = mybir.ActivationFunctionType

B, T, C = 2, 2048, 1024
H, HD = 16, 64
N_CORES = 8
G = 4               # head-quad parallel degree (within a batch)
HC = H // G         # heads per core (4)
PAIRS = HC // 2     # head pairs per core (2)
QC = 4              # query chunks
TQ = T // QC        # queries per chunk (512)
KT = T // 128       # key tiles (16)
CT = C // 128       # contraction tiles over C (8)
VW = HD + 1         # v columns per head incl. trailing ones column (65)
SCALE = 1.0 / np.sqrt(HD)

_CACHE = {}


def build_nc():
    nc = bacc.Bacc(None, target_bir_lowering=False, debug=False, num_devices=N_CORES)

    xT = nc.declare_dram_parameter("xT", [C, T], BF16, isOutput=False)
    wT = nc.declare_dram_parameter("wT", [C, 768], BF16, isOutput=False)
    wpT = nc.declare_dram_parameter("wpT", [256, C], BF16, isOutput=False)
    bqk = nc.declare_dram_parameter("bqk", [128, 4], F32, isOutput=False)
    out = nc.declare_dram_parameter("out", [C, T], BF16, isOutput=True)

    with tile.TileContext(nc) as tc:
        with tc.tile_pool(name="sb", bufs=1) as sb:
            # ---- persistent SBUF ----
            q_sb = [sb.tile([128, T], BF16, tag=f"q{p}", name=f"q{p}") for p in range(PAIRS)]
            k_sb = [sb.tile([128, T], BF16, tag=f"k{p}", name=f"k{p}") for p in range(PAIRS)]
            v_sb = [sb.tile([128, HC * VW], BF16, tag=f"v{t}", name=f"v{t}") for t in range(KT)]
            yn_sb = [sb.tile([128, T], BF16, tag=f"yn{p}", name=f"yn{p}") for p in range(PAIRS)]
            wpt_all = sb.tile([128, PAIRS, C], BF16, tag="wpt", name="wpt")
            bqk_sb = sb.tile([128, 4], F32, tag="bqk", name="bqk")
            ones_sb = sb.tile([1, HD], F32, tag="ones", name="ones")

            # x.T resident as one [128, CT, TQ] tile per 512-token chunk
            xtc = [sb.tile([128, CT, TQ], BF16, tag=f"xc{ch}", name=f"xc{ch}")
                   for ch in range(QC)]
            wtq = sb.tile([128, CT, 256], BF16, tag="wtq", name="wtq")
            wtk = sb.tile([128, CT, 256], BF16, tag="wtk", name="wtk")
            wtv = sb.tile([128, CT, 256], BF16, tag="wtv", name="wtv")

            nc.vector.memset(ones_sb[:, :], 1.0)
            for t in range(KT):
                vh = v_sb[t][:, :].rearrange("p (h c) -> p h c", c=VW)
                nc.vector.memset(vh[:, :, HD:HD + 1], 1.0)

            # 9 batched DMAs, in need order
            wTr = wT[:, :].rearrange("(k p) c -> p k c", p=128)
            nc.sync.dma_start(out=wtk[:, :, :], in_=wTr[:, :, 256:512])
            nc.sync.dma_start(out=wtv[:, :, :], in_=wTr[:, :, 512:768])
            nc.sync.dma_start(out=wtq[:, :, :], in_=wTr[:, :, 0:256])
            nc.sync.dma_start(out=bqk_sb[:, :], in_=bqk[:, :])
            xTr = xT[:, :].rearrange("(k p) t -> p k t", p=128)
            for ch in range(QC):
                nc.sync.dma_start(
                    out=xtc[ch][:, :, :], in_=xTr[:, :, TQ * ch:TQ * (ch + 1)]
                )
            nc.sync.dma_start(
                out=wpt_all[:, :, :],
                in_=wpT[:, :].rearrange("(j p) c -> p j c", p=128),
            )

            with tc.tile_pool(name="ps", bufs=1, space="PSUM") as ps:
                done = set()

                def emit_q(p, ch):
                    acc = ps.tile([128, TQ], F32, tag="lz", name="qacc", bufs=2)
                    for k in range(CT):
                        nc.tensor.matmul(
                            acc[:, :],
                            lhsT=wtq[:, k, 128 * p:128 * (p + 1)],
                            rhs=xtc[ch][:, k, :],
                            start=(k == 0), stop=(k == CT - 1),
                        )
                        yield None
                    nc.vector.tensor_scalar_add(
                        q_sb[p][:, TQ * ch:TQ * (ch + 1)], acc[:, :],
                        bqk_sb[:, p:p + 1],
                    )
                    done.add(("q", p, ch))

                def emit_k(p, ch):
                    acc = ps.tile([128, TQ], F32, tag="lz", name="kacc", bufs=2)
                    for k in range(CT):
                        nc.tensor.matmul(
                            acc[:, :],
                            lhsT=wtk[:, k, 128 * p:128 * (p + 1)],
                            rhs=xtc[ch][:, k, :],
                            start=(k == 0), stop=(k == CT - 1),
                        )
                        yield None
                    nc.vector.tensor_scalar_add(
                        k_sb[p][:, TQ * ch:TQ * (ch + 1)], acc[:, :],
                        bqk_sb[:, 2 + p:3 + p],
                    )
                    done.add(("k", p, ch))

                def emit_v(t):
                    # both pairs at once: N=256 amortizes the per-matmul overhead
                    acc = ps.tile([128, 256], F32, tag="lz", name="vacc", bufs=2)
                    ch, tt = t // 4, t % 4
                    for k in range(CT):
                        nc.tensor.matmul(
                            acc[:, :],
                            lhsT=xtc[ch][:, k, 128 * tt:128 * (tt + 1)],
                            rhs=wtv[:, k, :],
                            start=(k == 0), stop=(k == CT - 1),
                        )
                        yield None
                    vh = v_sb[t][:, :].rearrange("p (h c) -> p h c", c=VW)
                    nc.vector.tensor_copy(
                        vh[:, :, 0:HD],
                        acc[:, :].rearrange("p (h c) -> p h c", c=HD),
                    )
                    done.add(("v", t))

                # ---- eager: chunk-0 work only; the rest is demand-pulled ----
                for gen in (emit_k(0, 0), emit_q(0, 0),
                            emit_v(0), emit_v(1), emit_v(2), emit_v(3)):
                    for _ in gen:
                        pass

                def lazy_stream():
                    for ch in range(1, QC):
                        yield from emit_k(0, ch)
                        for t in range(4 * ch, 4 * ch + 4):
                            yield from emit_v(t)
                    for ch in range(1, QC):
                        yield from emit_q(0, ch)
                    for ch in range(QC):
                        yield from emit_k(1, ch)
                    for ch in range(QC):
                        yield from emit_q(1, ch)

                lz = lazy_stream()

                def pump(n):
                    for _ in range(n):
                        next(lz, None)

                def require(key):
                    while key not in done:
                        if next(lz, "END") == "END":
                            assert key in done, f"lazy stream ended before {key}"

                # ---- attention + deferred normalize + streaming proj ----
                sb_att = tc.alloc_tile_pool(name="sb_att", bufs=1)
                deferred = [None]
                proj_ready = []
                proj_queue = []

                def emit_normalize(item):
                    p, ch, yst, den = item
                    rc = sb_att.tile([1, 2 * TQ], F32, tag="recip", name="recip", bufs=2)
                    nc.vector.reciprocal_approx_fast(rc[:, :], den[:, :])
                    bcA = ps.tile([HD, TQ], F32, tag="lz", name="bcA", bufs=2)
                    bcB = ps.tile([HD, TQ], F32, tag="lz", name="bcB", bufs=2)
                    nc.tensor.matmul(
                        bcA[:, :], lhsT=ones_sb[:, :], rhs=rc[:, 0:TQ],
                        start=True, stop=True,
                    )
                    nc.tensor.matmul(
                        bcB[:, :], lhsT=ones_sb[:, :], rhs=rc[:, TQ:2 * TQ],
                        start=True, stop=True,
                    )
                    nc.vector.tensor_mul(
                        yn_sb[p][0:HD, TQ * ch:TQ * (ch + 1)],
                        yst[:, 0:TQ], bcA[:, :],
                    )
                    nc.vector.tensor_mul(
                        yn_sb[p][HD:128, TQ * ch:TQ * (ch + 1)],
                        yst[:, TQ:2 * TQ], bcB[:, :],
                    )
                    if p == PAIRS - 1:
                        proj_ready.append(ch)

                def emit_proj_chunk(ch):
                    for d in range(CT):
                        pacc = ps.tile([128, TQ], F32, tag="lz", name="pacc", bufs=2)
                        for p in range(PAIRS):
                            nc.tensor.matmul(
                                pacc[:, :],
                                lhsT=wpt_all[:, p, 128 * d:128 * (d + 1)],
                                rhs=yn_sb[p][:, TQ * ch:TQ * (ch + 1)],
                                start=(p == 0), stop=(p == PAIRS - 1),
                            )
                        otmp = sb_att.tile([128, TQ], BF16, tag="otmp", name="otmp", bufs=4)
                        nc.vector.tensor_copy(otmp[:, :], pacc[:, :])
                        nc.sync.dma_start(
                            out=out[128 * d:128 * (d + 1), TQ * ch:TQ * (ch + 1)],
                            in_=otmp[:, :],
                        )
                        yield None

                def pump_proj(n):
                    for _ in range(n):
                        if proj_queue:
                            if next(proj_queue[0], "END") == "END":
                                proj_queue.pop(0)

                for p in range(PAIRS):
                    for ch in range(QC):
                        require(("q", p, ch))
                        if p > 0:
                            for cc in range(QC):
                                require(("k", p, cc))
                        yab = ps.tile([VW, 2 * TQ], F32, tag="yab", name="yab")
                        for t in range(KT):
                            if p == 0 and ch == 0:
                                # demand-pull chunk t//4's k and this tile's v
                                if t % 4 == 0 and t > 0:
                                    require(("k", 0, t // 4))
                                require(("v", t))
                            sp = ps.tile([128, 2 * TQ], F32, tag="sp", name="sp", bufs=2)
                            nc.tensor.matmul(
                                sp[:, 0:TQ],
                                lhsT=k_sb[p][0:64, 128 * t:128 * (t + 1)],
                                rhs=q_sb[p][0:64, TQ * ch:TQ * (ch + 1)],
                                start=True, stop=True,
                            )
                            nc.tensor.matmul(
                                sp[:, TQ:2 * TQ],
                                lhsT=k_sb[p][64:128, 128 * t:128 * (t + 1)],
                                rhs=q_sb[p][64:128, TQ * ch:TQ * (ch + 1)],
                                start=True, stop=True,
                                tile_position=(64, 0),
                            )
                            pab = sb_att.tile([128, 2 * TQ], BF16, tag="pab", name="pab", bufs=4)
                            nc.scalar.activation(
                                pab[:, :], sp[:, :], AF.Exp, scale=float(SCALE),
                            )
                            vh = v_sb[t][:, :].rearrange("p (h c) -> p h c", c=VW)
                            nc.tensor.matmul(
                                yab[:, 0:TQ],
                                lhsT=vh[:, 2 * p, :],
                                rhs=pab[:, 0:TQ],
                                start=(t == 0), stop=(t == KT - 1),
                            )
                            nc.tensor.matmul(
                                yab[:, TQ:2 * TQ],
                                lhsT=vh[:, 2 * p + 1, :],
                                rhs=pab[:, TQ:2 * TQ],
                                start=(t == 0), stop=(t == KT - 1),
                            )
                            pump(2)
                            if t % 2 == 1:
                                pump_proj(1)
                            if t == 4 and deferred[0] is not None:
                                emit_normalize(deferred[0])
                                deferred[0] = None
                                while proj_ready:
                                    proj_queue.append(emit_proj_chunk(proj_ready.pop(0)))
                        yst = sb_att.tile([HD, 2 * TQ], F32, tag="yst", name="yst", bufs=2)
                        den = sb_att.tile([1, 2 * TQ], F32, tag="den", name="den", bufs=2)
                        nc.vector.tensor_copy(yst[:, :], yab[0:HD, :])
                        nc.vector.tensor_copy(den[0:1, :], yab[HD:VW, :])
                        deferred[0] = (p, ch, yst, den)

                # ---- tail: last normalize + remaining proj ----
                pump(10 ** 9)
                emit_normalize(deferred[0])
                while proj_ready:
                    proj_queue.append(emit_proj_chunk(proj_ready.pop(0)))
                pump_proj(10 ** 9)

                sb_att.release()

    nc.compile()
    return nc


def _get_nc():
    if "nc" not in _CACHE:
        _CACHE["nc"] = build_nc()
    return _CACHE["nc"]


def make_in_maps(x, W_attn, b_attn, W_proj, b_proj):
    x = np.asarray(x, dtype=np.float32)
    W_attn = np.asarray(W_attn, dtype=np.float32)
    b_attn = np.asarray(b_attn, dtype=np.float32)
    W_proj = np.asarray(W_proj, dtype=np.float32)

    bf = ml_dtypes.bfloat16
    xTg = [np.ascontiguousarray(x[g].T).astype(bf) for g in range(B)]  # [C, T]

    in_maps = []
    for c in range(N_CORES):
        g, hq = divmod(c, G)
        r0 = hq * 256  # first row of this core's q slice within W_attn[0:C]
        wslice = np.concatenate(
            [W_attn[r0:r0 + 256],                   # q rows
             W_attn[C + r0:C + r0 + 256],           # k rows
             W_attn[2 * C + r0:2 * C + r0 + 256]],  # v rows
            axis=0,
        )  # [768, C]
        wT = np.ascontiguousarray(wslice.T).astype(bf)  # [C, 768]
        wpT = np.ascontiguousarray(W_proj[:, r0:r0 + 256].T).astype(bf)  # [256, C]
        bq = b_attn[r0:r0 + 256].reshape(2, 128).T
        bk = b_attn[C + r0:C + r0 + 256].reshape(2, 128).T
        bqk_h = np.ascontiguousarray(
            np.concatenate([bq, bk], axis=1), dtype=np.float32
        )
        in_maps.append({
            "xT": xTg[g], "wT": wT, "wpT": wpT, "bqk": bqk_h,
        })
    return in_maps


def run_shards(in_maps, trace=False, **kw):
    nc = _get_nc()
    return run_bass_kernel_spmd(
        nc, in_maps, core_ids=list(range(N_CORES)), trace=trace, **kw
    )


def combine_outputs(res, W_proj, b_proj, b_attn):
    """Sum the 4 head-quad partial outputs per batch, add folded bias."""
    W_proj = np.asarray(W_proj, dtype=np.float32)
    b_proj = np.asarray(b_proj, dtype=np.float32)
    b_attn = np.asarray(b_attn, dtype=np.float32)
    b_adj = b_proj + W_proj @ b_attn[2 * C:]
    out = np.empty((B, T, C), dtype=np.float32)
    for g in range(B):
        acc = res.results[g * G + 0]["out"].astype(np.float32)
        for hq in range(1, G):
            acc = acc + res.results[g * G + hq]["out"].astype(np.float32)
        out[g] = acc.T + b_adj
    return out


def kernel(x, W_attn, b_attn, W_proj, b_proj):
    in_maps = make_in_maps(x, W_attn, b_attn, W_proj, b_proj)
    res = run_shards(in_maps)
    return combine_outputs(res, W_proj, b_proj, b_attn)


# revision 11
# speedup vs baseline: 1.3875x; 1.0043x over previous
"""Trainium2 Bass kernel for non-causal multi-head self-attention (B=2, T=2048,
C=1024, H=16, hd=64), SPMD over 8 NeuronCores.

Sharding: 2-way data parallel on batch x 4-way tensor parallel on heads.
Core c handles batch c//4 and heads [4*(c%4), 4*(c%4)+4) (= 2 head pairs)
for ALL 2048 tokens. c_attn is column-split, c_proj is row-split: each core
emits a PARTIAL output projection in bf16; the host unshard sums the 4
partials per batch and adds the (host-folded) bias. No device collectives.
This removes the 4x-redundant k/v projections of a seq-parallel split.

Structure / tricks:
- Host pre-arranges every input into the exact SBUF tile layout so each
  of the 9 input DMAs (and 4 output DMAs) is fully contiguous per
  partition (8KB+ runs). Strided/many-small DMAs cost ~600ns issue each
  on the Sync queue plus degraded transfer rate and were the dominant
  startup cost (~20-35us).
- x.T lands in four 512-token column chunks; attention unit (pair0,
  chunk0) starts as soon as chunk 0 + weights are in, with k(0,ch)/
  v(tiles) pulled on demand (require()) as later x chunks stream in.
- v for ALL 4 heads is produced by one matmul stream (N=256 instead of
  2x N=128: the PE has ~170ns fixed overhead per matmul).
- v stores a trailing ones-column per head; the PV matmul then yields
  softmax denominators as row 64 of y for free. The y-copy splits dims
  (rows 0:64) and denominator (row 64 -> partition 0 of a separate
  tile): vector.reciprocal_approx_fast mis-executes on partition offset
  64 (verified on HW), and engines require 32-aligned partition starts.
- No max-subtraction in softmax (logits ~N(0,1); exp is safe in fp32).
- Head-pair row-tiling: two K=64 S.T-matmuls run concurrently in PE row
  groups (0,0)/(64,0) writing one [128,1024] PSUM tile exp'd by a single
  ScalarE activation (ScalarE exp, 142us/core, is the bottleneck floor).
- PV runs ONE TILE LATE: the PE queue per tile is S(t),S(t),PV(t-1) so
  the in-order PE never head-of-line blocks on exp(t) (PV(t) depends on
  exp(t); emitting it before S(t+1) would serialize exp->PV->S->exp).
- The deferred normalize is split: approx-reciprocal at t==1 (DVE, runs
  while PE streams), PE ones-broadcast + DVE mults at t==6 when the
  reciprocal is long done - so the bc matmuls never stall the PE queue.
- Lazy qkv: q(0,1..3), k(1,*), q(1,*) stream into PE slack during
  attention (pump(3) on pair-0 units, pump(2) after), keeping the PE
  continuously busy (DVFS fast-clock) without require() bursts.
- proj contracts K=128 per pair; proj of query-chunk c is emitted as
  soon as both pairs' yn(c) exist (pumped every other key tile). Output
  staged in a [128, 8, 512] bf16 tile, one contiguous DMA per chunk.
- q/k biases via per-partition DVE tensor_scalar_add epilogues; v-bias
  and b_proj folded exactly into a host-side bias (softmax rows sum to
  1), added after the host reduction.
"""

import sys

for _p in ("/opt/trn_rl_repo",):
    if _p not in sys.path:
        sys.path.insert(0, _p)

import numpy as np
import ml_dtypes

import concourse.bass as bass
import concourse.mybir as mybir
import concourse.tile as tile
from concourse import bacc
from concourse.bass_utils import run_bass_kernel_spmd

BF16 = mybir.dt.bfloat16
F32 = mybir.dt.float32
AF = mybir.ActivationFunctionType

B, T, C = 2, 2048, 1024
H, HD = 16, 64
N_CORES = 8
G = 4               # head-quad parallel degree (within a batch)
HC = H // G         # heads per core (4)
PAIRS = HC // 2     # head pairs per core (2)
QC = 4              # query chunks
TQ = T // QC        # queries per chunk (512)
KT = T // 128       # key tiles (16)
CT = C // 128       # contraction tiles over C (8)
VW = HD + 1         # v columns per head incl. trailing ones column (65)
SCALE = 1.0 / np.sqrt(HD)

_CACHE = {}


def build_nc():
    nc = bacc.Bacc(None, target_bir_lowering=False, debug=False, num_devices=N_CORES)

    xc_d = [nc.declare_dram_parameter(f"xc{ch}", [128, CT * TQ], BF16, isOutput=False)
            for ch in range(QC)]
    wq_d = nc.declare_dram_parameter("wq", [128, CT * 256], BF16, isOutput=False)
    wk_d = nc.declare_dram_parameter("wk", [128, CT * 256], BF16, isOutput=False)
    wv_d = nc.declare_dram_parameter("wv", [128, CT * 256], BF16, isOutput=False)
    wp_d = nc.declare_dram_parameter("wp", [128, PAIRS * C], BF16, isOutput=False)
    bqk = nc.declare_dram_parameter("bqk", [128, 4], F32, isOutput=False)
    # out[p, ch, d, t] -> full[d*128+p, ch*512+t]
    out = nc.declare_dram_parameter("out", [128, QC * CT * TQ], BF16, isOutput=True)

    with tile.TileContext(nc) as tc:
        with tc.tile_pool(name="sb", bufs=1) as sb:
            # ---- persistent SBUF ----
            q_sb = [sb.tile([128, T], BF16, tag=f"q{p}", name=f"q{p}") for p in range(PAIRS)]
            k_sb = [sb.tile([128, T], BF16, tag=f"k{p}", name=f"k{p}") for p in range(PAIRS)]
            v_sb = [sb.tile([128, HC * VW], BF16, tag=f"v{t}", name=f"v{t}") for t in range(KT)]
            yn_sb = [sb.tile([128, T], BF16, tag=f"yn{p}", name=f"yn{p}") for p in range(PAIRS)]
            wpt_all = sb.tile([128, PAIRS, C], BF16, tag="wpt", name="wpt")
            bqk_sb = sb.tile([128, 4], F32, tag="bqk", name="bqk")
            ones_sb = sb.tile([1, HD], F32, tag="ones", name="ones")

            xtc = [sb.tile([128, CT, TQ], BF16, tag=f"xc{ch}", name=f"xc{ch}")
                   for ch in range(QC)]
            wtq = sb.tile([128, CT, 256], BF16, tag="wtq", name="wtq")
            wtk = sb.tile([128, CT, 256], BF16, tag="wtk", name="wtk")
            wtv = sb.tile([128, CT, 256], BF16, tag="wtv", name="wtv")

            nc.vector.memset(ones_sb[:, :], 1.0)
            for t in range(KT):
                vh = v_sb[t][:, :].rearrange("p (h c) -> p h c", c=VW)
                nc.vector.memset(vh[:, :, HD:HD + 1], 1.0)

            # batched contiguous DMAs, in need order
            def r3(ap, inner):
                return ap[:, :].rearrange("p (k c) -> p k c", c=inner)

            nc.sync.dma_start(out=wtk[:, :, :], in_=r3(wk_d, 256))
            nc.sync.dma_start(out=wtv[:, :, :], in_=r3(wv_d, 256))
            nc.sync.dma_start(out=wtq[:, :, :], in_=r3(wq_d, 256))
            nc.sync.dma_start(out=bqk_sb[:, :], in_=bqk[:, :])
            for ch in range(QC):
                nc.sync.dma_start(out=xtc[ch][:, :, :], in_=r3(xc_d[ch], TQ))
            nc.sync.dma_start(out=wpt_all[:, :, :], in_=r3(wp_d, C))

            with tc.tile_pool(name="ps", bufs=1, space="PSUM") as ps:
                done = set()

                def emit_q(p, ch):
                    acc = ps.tile([128, TQ], F32, tag="lz", name="qacc", bufs=2)
                    for k in range(CT):
                        nc.tensor.matmul(
                            acc[:, :],
                            lhsT=wtq[:, k, 128 * p:128 * (p + 1)],
                            rhs=xtc[ch][:, k, :],
                            start=(k == 0), stop=(k == CT - 1),
                        )
                        yield None
                    nc.vector.tensor_scalar_add(
                        q_sb[p][:, TQ * ch:TQ * (ch + 1)], acc[:, :],
                        bqk_sb[:, p:p + 1],
                    )
                    done.add(("q", p, ch))

                def emit_k(p, ch):
                    acc = ps.tile([128, TQ], F32, tag="lz", name="kacc", bufs=2)
                    for k in range(CT):
                        nc.tensor.matmul(
                            acc[:, :],
                            lhsT=wtk[:, k, 128 * p:128 * (p + 1)],
                            rhs=xtc[ch][:, k, :],
                            start=(k == 0), stop=(k == CT - 1),
                        )
                        yield None
                    nc.vector.tensor_scalar_add(
                        k_sb[p][:, TQ * ch:TQ * (ch + 1)], acc[:, :],
                        bqk_sb[:, 2 + p:3 + p],
                    )
                    done.add(("k", p, ch))

                def emit_v(t):
                    # both pairs at once: N=256 amortizes per-matmul overhead
                    acc = ps.tile([128, 256], F32, tag="lz", name="vacc", bufs=2)
                    ch, tt = t // 4, t % 4
                    for k in range(CT):
                        nc.tensor.matmul(
                            acc[:, :],
                            lhsT=xtc[ch][:, k, 128 * tt:128 * (tt + 1)],
                            rhs=wtv[:, k, :],
                            start=(k == 0), stop=(k == CT - 1),
                        )
                        yield None
                    vh = v_sb[t][:, :].rearrange("p (h c) -> p h c", c=VW)
                    nc.vector.tensor_copy(
                        vh[:, :, 0:HD],
                        acc[:, :].rearrange("p (h c) -> p h c", c=HD),
                    )
                    done.add(("v", t))

                # ---- eager: chunk-0 work only; the rest is demand-pulled ----
                for gen in (emit_k(0, 0), emit_q(0, 0),
                            emit_v(0), emit_v(1), emit_v(2), emit_v(3)):
                    for _ in gen:
                        pass

                def lazy_stream():
                    for ch in range(1, QC):
                        yield from emit_k(0, ch)
                        for t in range(4 * ch, 4 * ch + 4):
                            yield from emit_v(t)
                        yield from emit_q(0, ch)
                    for ch in range(QC):
                        yield from emit_k(1, ch)
                    for ch in range(QC):
                        yield from emit_q(1, ch)

                lz = lazy_stream()

                def pump(n):
                    for _ in range(n):
                        next(lz, None)

                def require(key):
                    while key not in done:
                        if next(lz, "END") == "END":
                            assert key in done, f"lazy stream ended before {key}"

                # ---- attention + split deferred normalize + streaming proj ----
                sb_att = tc.alloc_tile_pool(name="sb_att", bufs=1)
                deferred = [None]   # (p, ch, yst, den, rc)
                proj_ready = []
                proj_queue = []

                def emit_recip(item):
                    p, ch, yst, den = item
                    rc = sb_att.tile([1, 2 * TQ], F32, tag="recip", name="recip", bufs=2)
                    nc.vector.reciprocal_approx_fast(rc[:, :], den[:, :])
                    return (p, ch, yst, den, rc)

                def emit_bc_mults(item):
                    p, ch, yst, den, rc = item
                    bcA = ps.tile([HD, TQ], F32, tag="lz", name="bcA", bufs=2)
                    bcB = ps.tile([HD, TQ], F32, tag="lz", name="bcB", bufs=2)
                    nc.tensor.matmul(
                        bcA[:, :], lhsT=ones_sb[:, :], rhs=rc[:, 0:TQ],
                        start=True, stop=True,
                    )
                    nc.tensor.matmul(
                        bcB[:, :], lhsT=ones_sb[:, :], rhs=rc[:, TQ:2 * TQ],
                        start=True, stop=True,
                    )
                    nc.vector.tensor_mul(
                        yn_sb[p][0:HD, TQ * ch:TQ * (ch + 1)],
                        yst[:, 0:TQ], bcA[:, :],
                    )
                    nc.vector.tensor_mul(
                        yn_sb[p][HD:128, TQ * ch:TQ * (ch + 1)],
                        yst[:, TQ:2 * TQ], bcB[:, :],
                    )
                    if p == PAIRS - 1:
                        proj_ready.append(ch)

                out_r = out[:, :].rearrange(
                    "p (c d t) -> p c d t", c=QC, d=CT
                )

                def emit_proj_chunk(ch):
                    ot = sb_att.tile([128, CT, TQ], BF16, tag="otmp", name="otmp", bufs=2)
                    for d in range(CT):
                        pacc = ps.tile([128, TQ], F32, tag="lz", name="pacc", bufs=2)
                        for p in range(PAIRS):
                            nc.tensor.matmul(
                                pacc[:, :],
                                lhsT=wpt_all[:, p, 128 * d:128 * (d + 1)],
                                rhs=yn_sb[p][:, TQ * ch:TQ * (ch + 1)],
                                start=(p == 0), stop=(p == PAIRS - 1),
                            )
                        nc.vector.tensor_copy(ot[:, d, :], pacc[:, :])
                        yield None
                    nc.sync.dma_start(out=out_r[:, ch, :, :], in_=ot[:, :, :])

                def pump_proj(n):
                    for _ in range(n):
                        if proj_queue:
                            if next(proj_queue[0], "END") == "END":
                                proj_queue.pop(0)

                for p in range(PAIRS):
                    for ch in range(QC):
                        require(("q", p, ch))
                        if p > 0:
                            for cc in range(QC):
                                require(("k", p, cc))
                        yab = ps.tile([VW, 2 * TQ], F32, tag="yab", name="yab")
                        prev = [None]

                        def emit_pv(tt, pab):
                            if p == 0 and ch == 0:
                                require(("v", tt))
                            vh = v_sb[tt][:, :].rearrange("p (h c) -> p h c", c=VW)
                            nc.tensor.matmul(
                                yab[:, 0:TQ],
                                lhsT=vh[:, 2 * p, :],
                                rhs=pab[:, 0:TQ],
                                start=(tt == 0), stop=(tt == KT - 1),
                            )
                            nc.tensor.matmul(
                                yab[:, TQ:2 * TQ],
                                lhsT=vh[:, 2 * p + 1, :],
                                rhs=pab[:, TQ:2 * TQ],
                                start=(tt == 0), stop=(tt == KT - 1),
                            )

                        for t in range(KT):
                            if p == 0 and ch == 0 and t % 4 == 0 and t > 0:
                                require(("k", 0, t // 4))
                            sp = ps.tile([128, 2 * TQ], F32, tag="sp", name="sp", bufs=2)
                            nc.tensor.matmul(
                                sp[:, 0:TQ],
                                lhsT=k_sb[p][0:64, 128 * t:128 * (t + 1)],
                                rhs=q_sb[p][0:64, TQ * ch:TQ * (ch + 1)],
                                start=True, stop=True,
                            )
                            nc.tensor.matmul(
                                sp[:, TQ:2 * TQ],
                                lhsT=k_sb[p][64:128, 128 * t:128 * (t + 1)],
                                rhs=q_sb[p][64:128, TQ * ch:TQ * (ch + 1)],
                                start=True, stop=True,
                                tile_position=(64, 0),
                            )
                            pab = sb_att.tile([128, 2 * TQ], BF16, tag="pab", name="pab", bufs=4)
                            nc.scalar.activation(
                                pab[:, :], sp[:, :], AF.Exp, scale=float(SCALE),
                            )
                            if prev[0] is not None:
                                emit_pv(*prev[0])
                            prev[0] = (t, pab)
                            pump(3 if p == 0 else 2)
                            if t % 2 == 1:
                                pump_proj(1)
                            if t == 1 and deferred[0] is not None:
                                deferred[0] = emit_recip(deferred[0])
                            if t == 6 and deferred[0] is not None:
                                emit_bc_mults(deferred[0])
                                deferred[0] = None
                                while proj_ready:
                                    proj_queue.append(emit_proj_chunk(proj_ready.pop(0)))
                        emit_pv(*prev[0])
                        yst = sb_att.tile([HD, 2 * TQ], F32, tag="yst", name="yst", bufs=2)
                        den = sb_att.tile([1, 2 * TQ], F32, tag="den", name="den", bufs=2)
                        nc.vector.tensor_copy(yst[:, :], yab[0:HD, :])
                        nc.vector.tensor_copy(den[0:1, :], yab[HD:VW, :])
                        deferred[0] = (p, ch, yst, den)

                # ---- tail: last normalize + remaining proj ----
                pump(10 ** 9)
                emit_bc_mults(emit_recip(deferred[0]))
                while proj_ready:
                    proj_queue.append(emit_proj_chunk(proj_ready.pop(0)))
                pump_proj(10 ** 9)

                sb_att.release()

    nc.compile()
    return nc


def _get_nc():
    if "nc" not in _CACHE:
        _CACHE["nc"] = build_nc()
    return _CACHE["nc"]


def _to_tiles(a, nblk, blk):
    """[nblk*blk, F] -> [blk, nblk*F] with row p holding all nblk blocks."""
    F = a.shape[1]
    return np.ascontiguousarray(
        a.reshape(nblk, blk, F).transpose(1, 0, 2).reshape(blk, nblk * F)
    )


def make_in_maps(x, W_attn, b_attn, W_proj, b_proj):
    x = np.asarray(x, dtype=np.float32)
    W_attn = np.asarray(W_attn, dtype=np.float32)
    b_attn = np.asarray(b_attn, dtype=np.float32)
    W_proj = np.asarray(W_proj, dtype=np.float32)

    bf = ml_dtypes.bfloat16
    xTg = [np.ascontiguousarray(x[g].T).astype(bf) for g in range(B)]  # [C, T]

    in_maps = []
    for c in range(N_CORES):
        g, hq = divmod(c, G)
        r0 = hq * 256  # first row of this core's q slice within W_attn[0:C]
        wq = _to_tiles(W_attn[r0:r0 + 256].T.astype(bf), CT, 128)
        wk = _to_tiles(W_attn[C + r0:C + r0 + 256].T.astype(bf), CT, 128)
        wv = _to_tiles(W_attn[2 * C + r0:2 * C + r0 + 256].T.astype(bf), CT, 128)
        wp = _to_tiles(W_proj[:, r0:r0 + 256].T.astype(bf), PAIRS, 128)
        bq = b_attn[r0:r0 + 256].reshape(2, 128).T
        bk = b_attn[C + r0:C + r0 + 256].reshape(2, 128).T
        bqk_h = np.ascontiguousarray(
            np.concatenate([bq, bk], axis=1), dtype=np.float32
        )
        m = {"wq": wq, "wk": wk, "wv": wv, "wp": wp, "bqk": bqk_h}
        for ch in range(QC):
            m[f"xc{ch}"] = _to_tiles(xTg[g][:, TQ * ch:TQ * (ch + 1)], CT, 128)
        in_maps.append(m)
    return in_maps


def run_shards(in_maps, trace=False, **kw):
    nc = _get_nc()
    return run_bass_kernel_spmd(
        nc, in_maps, core_ids=list(range(N_CORES)), trace=trace, **kw
    )


def combine_outputs(res, W_proj, b_proj, b_attn):
    """Sum the 4 head-quad partial outputs per batch, add folded bias."""
    W_proj = np.asarray(W_proj, dtype=np.float32)
    b_proj = np.asarray(b_proj, dtype=np.float32)
    b_attn = np.asarray(b_attn, dtype=np.float32)
    b_adj = b_proj + W_proj @ b_attn[2 * C:]
    out = np.empty((B, T, C), dtype=np.float32)
    for g in range(B):
        acc = None
        for hq in range(G):
            r = res.results[g * G + hq]["out"].astype(np.float32)
            # r[p, ch, d, t] -> [d*128+p, ch*512+t]
            r = r.reshape(128, QC, CT, TQ).transpose(2, 0, 1, 3).reshape(C, T)
            acc = r if acc is None else acc + r
        out[g] = acc.T + b_adj
    return out


def kernel(x, W_attn, b_attn, W_proj, b_proj):
    in_maps = make_in_maps(x, W_attn, b_attn, W_proj, b_proj)
    res = run_shards(in_maps)
    return combine_outputs(res, W_proj, b_proj, b_attn)


# revision 12
# speedup vs baseline: 1.3965x; 1.0065x over previous
"""Trainium2 Bass kernel for non-causal multi-head self-attention (B=2, T=2048,
C=1024, H=16, hd=64), SPMD over 8 NeuronCores.

Sharding: 2-way data parallel on batch x 4-way tensor parallel on heads.
Core c handles batch c//4 and heads [4*(c%4), 4*(c%4)+4) (= 2 head pairs)
for ALL 2048 tokens. c_attn is column-split, c_proj is row-split: each core
emits a PARTIAL output projection in bf16; the host unshard sums the 4
partials per batch and adds the (host-folded) bias. No device collectives.
This removes the 4x-redundant k/v projections of a seq-parallel split.

Structure / tricks:
- Host pre-arranges every input into the exact SBUF tile layout so each
  of the 9 input DMAs (and 4 output DMAs) is fully contiguous per
  partition (8KB+ runs). Strided/many-small DMAs cost ~600ns issue each
  on the Sync queue plus degraded transfer rate and were the dominant
  startup cost (~20-35us).
- x.T lands in four 512-token column chunks; attention unit (pair0,
  chunk0) starts as soon as chunk 0 + weights are in, with k(0,ch)/
  v(tiles) pulled on demand (require()) as later x chunks stream in.
- v for ALL 4 heads is produced by one matmul stream (N=256 instead of
  2x N=128: the PE has ~170ns fixed overhead per matmul).
- v stores a trailing ones-column per head; the PV matmul then yields
  softmax denominators as row 64 of y for free. The y-copy splits dims
  (rows 0:64) and denominator (row 64 -> partition 0 of a separate
  tile): vector.reciprocal_approx_fast mis-executes on partition offset
  64 (verified on HW), and engines require 32-aligned partition starts.
- No max-subtraction in softmax (logits ~N(0,1); exp is safe in fp32).
- Head-pair row-tiling: two K=64 S.T-matmuls run concurrently in PE row
  groups (0,0)/(64,0) writing one [128,1024] PSUM tile exp'd by a single
  ScalarE activation (ScalarE exp, 142us/core, is the bottleneck floor).
- PV runs ONE TILE LATE: the PE queue per tile is S(t),S(t),PV(t-1) so
  the in-order PE never head-of-line blocks on exp(t) (PV(t) depends on
  exp(t); emitting it before S(t+1) would serialize exp->PV->S->exp).
- The deferred normalize is split: approx-reciprocal at t==1 (DVE, runs
  while PE streams), PE ones-broadcast + DVE mults at t==6 when the
  reciprocal is long done - so the bc matmuls never stall the PE queue.
- Lazy qkv: q(0,1..3), k(1,*), q(1,*) stream into PE slack during
  attention (pump(3) on pair-0 units, pump(2) after), keeping the PE
  continuously busy (DVFS fast-clock) without require() bursts.
- proj contracts K=128 per pair; proj of query-chunk c is emitted as
  soon as both pairs' yn(c) exist (pumped every other key tile). Output
  staged in a [128, 8, 512] bf16 tile, one contiguous DMA per chunk.
- q/k biases via per-partition DVE tensor_scalar_add epilogues; v-bias
  and b_proj folded exactly into a host-side bias (softmax rows sum to
  1), added after the host reduction.
"""

import sys

for _p in ("/opt/trn_rl_repo",):
    if _p not in sys.path:
        sys.path.insert(0, _p)

import numpy as np
import ml_dtypes

import concourse.bass as bass
import concourse.mybir as mybir
import concourse.tile as tile
from concourse import bacc
from concourse.bass_utils import run_bass_kernel_spmd

BF16 = mybir.dt.bfloat16
F32 = mybir.dt.float32
AF = mybir.ActivationFunctionType

B, T, C = 2, 2048, 1024
H, HD = 16, 64
N_CORES = 8
G = 4               # head-quad parallel degree (within a batch)
HC = H // G         # heads per core (4)
PAIRS = HC // 2     # head pairs per core (2)
QC = 4              # query chunks
TQ = T // QC        # queries per chunk (512)
KT = T // 128       # key tiles (16)
CT = C // 128       # contraction tiles over C (8)
VW = HD + 1         # v columns per head incl. trailing ones column (65)
SCALE = 1.0 / np.sqrt(HD)

_CACHE = {}


def build_nc():
    nc = bacc.Bacc(None, target_bir_lowering=False, debug=False, num_devices=N_CORES)

    xc_d = [nc.declare_dram_parameter(f"xc{ch}", [128, CT * TQ], BF16, isOutput=False)
            for ch in range(QC)]
    wq_d = nc.declare_dram_parameter("wq", [128, CT * 256], BF16, isOutput=False)
    wk_d = nc.declare_dram_parameter("wk", [128, CT * 256], BF16, isOutput=False)
    wv_d = nc.declare_dram_parameter("wv", [128, CT * 256], BF16, isOutput=False)
    wp_d = nc.declare_dram_parameter("wp", [128, PAIRS * C], BF16, isOutput=False)
    bqk = nc.declare_dram_parameter("bqk", [128, 4], F32, isOutput=False)
    # out[p, ch, d, t] -> full[d*128+p, ch*512+t]
    out = nc.declare_dram_parameter("out", [128, QC * CT * TQ], BF16, isOutput=True)

    with tile.TileContext(nc) as tc:
        with tc.tile_pool(name="sb", bufs=1) as sb:
            # ---- persistent SBUF ----
            q_sb = [sb.tile([128, T], BF16, tag=f"q{p}", name=f"q{p}") for p in range(PAIRS)]
            k_sb = [sb.tile([128, T], BF16, tag=f"k{p}", name=f"k{p}") for p in range(PAIRS)]
            v_sb = [sb.tile([128, HC * VW], BF16, tag=f"v{t}", name=f"v{t}") for t in range(KT)]
            yn_sb = [sb.tile([128, T], BF16, tag=f"yn{p}", name=f"yn{p}") for p in range(PAIRS)]
            wpt_all = sb.tile([128, PAIRS, C], BF16, tag="wpt", name="wpt")
            bqk_sb = sb.tile([128, 4], F32, tag="bqk", name="bqk")
            ones_sb = sb.tile([1, HD], F32, tag="ones", name="ones")

            xtc = [sb.tile([128, CT, TQ], BF16, tag=f"xc{ch}", name=f"xc{ch}")
                   for ch in range(QC)]
            wtq = sb.tile([128, CT, 256], BF16, tag="wtq", name="wtq")
            wtk = sb.tile([128, CT, 256], BF16, tag="wtk", name="wtk")
            wtv = sb.tile([128, CT, 256], BF16, tag="wtv", name="wtv")

            nc.vector.memset(ones_sb[:, :], 1.0)
            for t in range(KT):
                vh = v_sb[t][:, :].rearrange("p (h c) -> p h c", c=VW)
                nc.vector.memset(vh[:, :, HD:HD + 1], 1.0)

            # batched contiguous DMAs, in need order
            def r3(ap, inner):
                return ap[:, :].rearrange("p (k c) -> p k c", c=inner)

            nc.scalar.dma_start(out=wtk[:, :, :], in_=r3(wk_d, 256))
            nc.scalar.dma_start(out=wtv[:, :, :], in_=r3(wv_d, 256))
            nc.scalar.dma_start(out=wtq[:, :, :], in_=r3(wq_d, 256))
            nc.scalar.dma_start(out=bqk_sb[:, :], in_=bqk[:, :])
            for ch in range(QC):
                nc.sync.dma_start(out=xtc[ch][:, :, :], in_=r3(xc_d[ch], TQ))
            nc.scalar.dma_start(out=wpt_all[:, :, :], in_=r3(wp_d, C))

            with tc.tile_pool(name="ps", bufs=1, space="PSUM") as ps:
                done = set()

                def emit_q(p, ch):
                    acc = ps.tile([128, TQ], F32, tag="lz", name="qacc", bufs=2)
                    for k in range(CT):
                        nc.tensor.matmul(
                            acc[:, :],
                            lhsT=wtq[:, k, 128 * p:128 * (p + 1)],
                            rhs=xtc[ch][:, k, :],
                            start=(k == 0), stop=(k == CT - 1),
                        )
                        yield None
                    nc.vector.tensor_scalar_add(
                        q_sb[p][:, TQ * ch:TQ * (ch + 1)], acc[:, :],
                        bqk_sb[:, p:p + 1],
                    )
                    done.add(("q", p, ch))

                def emit_k(p, ch):
                    acc = ps.tile([128, TQ], F32, tag="lz", name="kacc", bufs=2)
                    for k in range(CT):
                        nc.tensor.matmul(
                            acc[:, :],
                            lhsT=wtk[:, k, 128 * p:128 * (p + 1)],
                            rhs=xtc[ch][:, k, :],
                            start=(k == 0), stop=(k == CT - 1),
                        )
                        yield None
                    nc.vector.tensor_scalar_add(
                        k_sb[p][:, TQ * ch:TQ * (ch + 1)], acc[:, :],
                        bqk_sb[:, 2 + p:3 + p],
                    )
                    done.add(("k", p, ch))

                def emit_v(t):
                    # both pairs at once: N=256 amortizes per-matmul overhead
                    acc = ps.tile([128, 256], F32, tag="lz", name="vacc", bufs=2)
                    ch, tt = t // 4, t % 4
                    for k in range(CT):
                        nc.tensor.matmul(
                            acc[:, :],
                            lhsT=xtc[ch][:, k, 128 * tt:128 * (tt + 1)],
                            rhs=wtv[:, k, :],
                            start=(k == 0), stop=(k == CT - 1),
                        )
                        yield None
                    vh = v_sb[t][:, :].rearrange("p (h c) -> p h c", c=VW)
                    nc.vector.tensor_copy(
                        vh[:, :, 0:HD],
                        acc[:, :].rearrange("p (h c) -> p h c", c=HD),
                    )
                    done.add(("v", t))

                # ---- eager: chunk-0 work only; the rest is demand-pulled ----
                for gen in (emit_k(0, 0), emit_q(0, 0),
                            emit_v(0), emit_v(1), emit_v(2), emit_v(3)):
                    for _ in gen:
                        pass

                def lazy_stream():
                    for ch in range(1, QC):
                        yield from emit_k(0, ch)
                        for t in range(4 * ch, 4 * ch + 4):
                            yield from emit_v(t)
                        yield from emit_q(0, ch)
                    for ch in range(QC):
                        yield from emit_k(1, ch)
                    for ch in range(QC):
                        yield from emit_q(1, ch)

                lz = lazy_stream()

                def pump(n):
                    for _ in range(n):
                        next(lz, None)

                def require(key):
                    while key not in done:
                        if next(lz, "END") == "END":
                            assert key in done, f"lazy stream ended before {key}"

                # ---- attention + split deferred normalize + streaming proj ----
                sb_att = tc.alloc_tile_pool(name="sb_att", bufs=1)
                deferred = [None]   # (p, ch, yst, den, rc)
                proj_ready = []
                proj_queue = []

                def emit_recip(item):
                    p, ch, yst, den = item
                    rc = sb_att.tile([1, 2 * TQ], F32, tag="recip", name="recip", bufs=2)
                    nc.vector.reciprocal_approx_fast(rc[:, :], den[:, :])
                    return (p, ch, yst, den, rc)

                def emit_bc_mults(item):
                    p, ch, yst, den, rc = item
                    bcA = ps.tile([HD, TQ], F32, tag="lz", name="bcA", bufs=2)
                    bcB = ps.tile([HD, TQ], F32, tag="lz", name="bcB", bufs=2)
                    nc.tensor.matmul(
                        bcA[:, :], lhsT=ones_sb[:, :], rhs=rc[:, 0:TQ],
                        start=True, stop=True,
                    )
                    nc.tensor.matmul(
                        bcB[:, :], lhsT=ones_sb[:, :], rhs=rc[:, TQ:2 * TQ],
                        start=True, stop=True,
                    )
                    nc.vector.tensor_mul(
                        yn_sb[p][0:HD, TQ * ch:TQ * (ch + 1)],
                        yst[:, 0:TQ], bcA[:, :],
                    )
                    nc.vector.tensor_mul(
                        yn_sb[p][HD:128, TQ * ch:TQ * (ch + 1)],
                        yst[:, TQ:2 * TQ], bcB[:, :],
                    )
                    if p == PAIRS - 1:
                        proj_ready.append(ch)

                out_r = out[:, :].rearrange(
                    "p (c d t) -> p c d t", c=QC, d=CT
                )

                def emit_proj_chunk(ch):
                    ot = sb_att.tile([128, CT, TQ], BF16, tag="otmp", name="otmp", bufs=2)
                    eng = nc.scalar if ch == QC - 1 else nc.sync
                    for d in range(CT):
                        pacc = ps.tile([128, TQ], F32, tag="lz", name="pacc", bufs=2)
                        for p in range(PAIRS):
                            nc.tensor.matmul(
                                pacc[:, :],
                                lhsT=wpt_all[:, p, 128 * d:128 * (d + 1)],
                                rhs=yn_sb[p][:, TQ * ch:TQ * (ch + 1)],
                                start=(p == 0), stop=(p == PAIRS - 1),
                            )
                        nc.vector.tensor_copy(ot[:, d, :], pacc[:, :])
                        if d == CT // 2 - 1:
                            eng.dma_start(
                                out=out_r[:, ch, 0:CT // 2, :],
                                in_=ot[:, 0:CT // 2, :],
                            )
                        yield None
                    eng.dma_start(
                        out=out_r[:, ch, CT // 2:CT, :],
                        in_=ot[:, CT // 2:CT, :],
                    )

                def pump_proj(n):
                    for _ in range(n):
                        if proj_queue:
                            if next(proj_queue[0], "END") == "END":
                                proj_queue.pop(0)

                for p in range(PAIRS):
                    for ch in range(QC):
                        require(("q", p, ch))
                        if p > 0:
                            for cc in range(QC):
                                require(("k", p, cc))
                        yab = ps.tile([VW, 2 * TQ], F32, tag="yab", name="yab")
                        prev = [None]

                        def emit_pv(tt, pab):
                            if p == 0 and ch == 0:
                                require(("v", tt))
                            vh = v_sb[tt][:, :].rearrange("p (h c) -> p h c", c=VW)
                            nc.tensor.matmul(
                                yab[:, 0:TQ],
                                lhsT=vh[:, 2 * p, :],
                                rhs=pab[:, 0:TQ],
                                start=(tt == 0), stop=(tt == KT - 1),
                            )
                            nc.tensor.matmul(
                                yab[:, TQ:2 * TQ],
                                lhsT=vh[:, 2 * p + 1, :],
                                rhs=pab[:, TQ:2 * TQ],
                                start=(tt == 0), stop=(tt == KT - 1),
                            )

                        for t in range(KT):
                            if p == 0 and ch == 0 and t % 4 == 0 and t > 0:
                                require(("k", 0, t // 4))
                            sp = ps.tile([128, 2 * TQ], F32, tag="sp", name="sp", bufs=2)
                            nc.tensor.matmul(
                                sp[:, 0:TQ],
                                lhsT=k_sb[p][0:64, 128 * t:128 * (t + 1)],
                                rhs=q_sb[p][0:64, TQ * ch:TQ * (ch + 1)],
                                start=True, stop=True,
                            )
                            nc.tensor.matmul(
                                sp[:, TQ:2 * TQ],
                                lhsT=k_sb[p][64:128, 128 * t:128 * (t + 1)],
                                rhs=q_sb[p][64:128, TQ * ch:TQ * (ch + 1)],
                                start=True, stop=True,
                                tile_position=(64, 0),
                            )
                            pab = sb_att.tile([128, 2 * TQ], BF16, tag="pab", name="pab", bufs=4)
                            nc.scalar.activation(
                                pab[:, :], sp[:, :], AF.Exp, scale=float(SCALE),
                            )
                            if prev[0] is not None:
                                emit_pv(*prev[0])
                            prev[0] = (t, pab)
                            pump(6 if (p == 0 and ch == 0) else 3 if p == 0 else 2)
                            if t % 2 == 1:
                                pump_proj(1)
                            if t == 1 and deferred[0] is not None:
                                deferred[0] = emit_recip(deferred[0])
                            if t == 6 and deferred[0] is not None:
                                emit_bc_mults(deferred[0])
                                deferred[0] = None
                                while proj_ready:
                                    proj_queue.append(emit_proj_chunk(proj_ready.pop(0)))
                        emit_pv(*prev[0])
                        yst = sb_att.tile([HD, 2 * TQ], F32, tag="yst", name="yst", bufs=2)
                        den = sb_att.tile([1, 2 * TQ], F32, tag="den", name="den", bufs=2)
                        nc.vector.tensor_copy(yst[:, :], yab[0:HD, :])
                        nc.vector.tensor_copy(den[0:1, :], yab[HD:VW, :])
                        deferred[0] = (p, ch, yst, den)

                # ---- tail: last normalize + remaining proj ----
                pump(10 ** 9)
                emit_bc_mults(emit_recip(deferred[0]))
                while proj_ready:
                    proj_queue.append(emit_proj_chunk(proj_ready.pop(0)))
                pump_proj(10 ** 9)

                sb_att.release()

    nc.compile()
    return nc


def _get_nc():
    if "nc" not in _CACHE:
        _CACHE["nc"] = build_nc()
    return _CACHE["nc"]


def _to_tiles(a, nblk, blk):
    """[nblk*blk, F] -> [blk, nblk*F] with row p holding all nblk blocks."""
    F = a.shape[1]
    return np.ascontiguousarray(
        a.reshape(nblk, blk, F).transpose(1, 0, 2).reshape(blk, nblk * F)
    )


def make_in_maps(x, W_attn, b_attn, W_proj, b_proj):
    x = np.asarray(x, dtype=np.float32)
    W_attn = np.asarray(W_attn, dtype=np.float32)
    b_attn = np.asarray(b_attn, dtype=np.float32)
    W_proj = np.asarray(W_proj, dtype=np.float32)

    bf = ml_dtypes.bfloat16
    xTg = [np.ascontiguousarray(x[g].T).astype(bf) for g in range(B)]  # [C, T]

    in_maps = []
    for c in range(N_CORES):
        g, hq = divmod(c, G)
        r0 = hq * 256  # first row of this core's q slice within W_attn[0:C]
        wq = _to_tiles(W_attn[r0:r0 + 256].T.astype(bf), CT, 128)
        wk = _to_tiles(W_attn[C + r0:C + r0 + 256].T.astype(bf), CT, 128)
        wv = _to_tiles(W_attn[2 * C + r0:2 * C + r0 + 256].T.astype(bf), CT, 128)
        wp = _to_tiles(W_proj[:, r0:r0 + 256].T.astype(bf), PAIRS, 128)
        bq = b_attn[r0:r0 + 256].reshape(2, 128).T
        bk = b_attn[C + r0:C + r0 + 256].reshape(2, 128).T
        bqk_h = np.ascontiguousarray(
            np.concatenate([bq, bk], axis=1), dtype=np.float32
        )
        m = {"wq": wq, "wk": wk, "wv": wv, "wp": wp, "bqk": bqk_h}
        for ch in range(QC):
            m[f"xc{ch}"] = _to_tiles(xTg[g][:, TQ * ch:TQ * (ch + 1)], CT, 128)
        in_maps.append(m)
    return in_maps


def run_shards(in_maps, trace=False, **kw):
    nc = _get_nc()
    return run_bass_kernel_spmd(
        nc, in_maps, core_ids=list(range(N_CORES)), trace=trace, **kw
    )


def combine_outputs(res, W_proj, b_proj, b_attn):
    """Sum the 4 head-quad partial outputs per batch, add folded bias."""
    W_proj = np.asarray(W_proj, dtype=np.float32)
    b_proj = np.asarray(b_proj, dtype=np.float32)
    b_attn = np.asarray(b_attn, dtype=np.float32)
    b_adj = b_proj + W_proj @ b_attn[2 * C:]
    out = np.empty((B, T, C), dtype=np.float32)
    for g in range(B):
        acc = None
        for hq in range(G):
            r = res.results[g * G + hq]["out"].astype(np.float32)
            # r[p, ch, d, t] -> [d*128+p, ch*512+t]
            r = r.reshape(128, QC, CT, TQ).transpose(2, 0, 1, 3).reshape(C, T)
            acc = r if acc is None else acc + r
        out[g] = acc.T + b_adj
    return out


def kernel(x, W_attn, b_attn, W_proj, b_proj):
    in_maps = make_in_maps(x, W_attn, b_attn, W_proj, b_proj)
    res = run_shards(in_maps)
    return combine_outputs(res, W_proj, b_proj, b_attn)


# revision 13
# speedup vs baseline: 1.4521x; 1.0398x over previous
"""Trainium2 Bass kernel for non-causal multi-head self-attention (B=2, T=2048,
C=1024, H=16, hd=64), SPMD over 8 NeuronCores.

Sharding: 2-way data parallel on batch x 4-way tensor parallel on heads.
Core c handles batch c//4 and heads [4*(c%4), 4*(c%4)+4) (= 2 head pairs)
for ALL 2048 tokens. c_attn is column-split, c_proj is row-split: each core
emits a PARTIAL output projection in bf16; the host unshard sums the 4
partials per batch and adds the (host-folded) bias. No device collectives.
This removes the 4x-redundant k/v projections of a seq-parallel split.

Structure / tricks:
- Host pre-arranges every input into the exact SBUF tile layout so each
  of the 9 input DMAs (and 4 output DMAs) is fully contiguous per
  partition (8KB+ runs). Strided/many-small DMAs cost ~600ns issue each
  on the Sync queue plus degraded transfer rate and were the dominant
  startup cost (~20-35us).
- x.T lands in four 512-token column chunks; attention unit (pair0,
  chunk0) starts as soon as chunk 0 + weights are in, with k(0,ch)/
  v(tiles) pulled on demand (require()) as later x chunks stream in.
- v for ALL 4 heads is produced by one matmul stream (N=256 instead of
  2x N=128: the PE has ~170ns fixed overhead per matmul).
- v stores a trailing ones-column per head; the PV matmul then yields
  softmax denominators as row 64 of y for free. The y-copy splits dims
  (rows 0:64) and denominator (row 64 -> partition 0 of a separate
  tile): vector.reciprocal_approx_fast mis-executes on partition offset
  64 (verified on HW), and engines require 32-aligned partition starts.
- No max-subtraction in softmax (logits ~N(0,1); exp is safe in fp32).
- Head-pair row-tiling: two K=64 S.T-matmuls run concurrently in PE row
  groups (0,0)/(64,0) writing one [128,1024] PSUM tile exp'd by a single
  ScalarE activation (ScalarE exp, 142us/core, is the bottleneck floor).
- PV runs ONE TILE LATE: the PE queue per tile is S(t),S(t),PV(t-1) so
  the in-order PE never head-of-line blocks on exp(t) (PV(t) depends on
  exp(t); emitting it before S(t+1) would serialize exp->PV->S->exp).
- The deferred normalize is split: approx-reciprocal at t==1 (DVE, runs
  while PE streams), PE ones-broadcast + DVE mults at t==6 when the
  reciprocal is long done - so the bc matmuls never stall the PE queue.
- Lazy qkv: q(0,1..3), k(1,*), q(1,*) stream into PE slack during
  attention (pump(3) on pair-0 units, pump(2) after), keeping the PE
  continuously busy (DVFS fast-clock) without require() bursts.
- proj contracts K=128 per pair; proj of query-chunk c is emitted as
  soon as both pairs' yn(c) exist (pumped every other key tile). Output
  staged in a [128, 8, 512] bf16 tile, one contiguous DMA per chunk.
- q/k biases via per-partition DVE tensor_scalar_add epilogues; v-bias
  and b_proj folded exactly into a host-side bias (softmax rows sum to
  1), added after the host reduction.
"""

import sys

for _p in ("/opt/trn_rl_repo",):
    if _p not in sys.path:
        sys.path.insert(0, _p)

import numpy as np
import ml_dtypes

import concourse.bass as bass
import concourse.mybir as mybir
import concourse.tile as tile
from concourse import bacc
from concourse.bass_utils import run_bass_kernel_spmd

BF16 = mybir.dt.bfloat16
F32 = mybir.dt.float32
AF = mybir.ActivationFunctionType

B, T, C = 2, 2048, 1024
H, HD = 16, 64
N_CORES = 8
G = 4               # head-quad parallel degree (within a batch)
HC = H // G         # heads per core (4)
PAIRS = HC // 2     # head pairs per core (2)
QC = 4              # query chunks
TQ = T // QC        # queries per chunk (512)
KT = T // 128       # key tiles (16)
CT = C // 128       # contraction tiles over C (8)
VW = HD + 1         # v columns per head incl. trailing ones column (65)
SCALE = 1.0 / np.sqrt(HD)

_CACHE = {}


def build_nc():
    nc = bacc.Bacc(None, target_bir_lowering=False, debug=False, num_devices=N_CORES)

    xc_d = [nc.declare_dram_parameter(f"xc{ch}", [128, CT * TQ], BF16, isOutput=False)
            for ch in range(QC)]
    wq_d = nc.declare_dram_parameter("wq", [128, CT * 256], BF16, isOutput=False)
    wk_d = nc.declare_dram_parameter("wk", [128, CT * 256], BF16, isOutput=False)
    wv_d = nc.declare_dram_parameter("wv", [128, CT * 256], BF16, isOutput=False)
    wp_d = nc.declare_dram_parameter("wp", [128, PAIRS * C], BF16, isOutput=False)
    bqk = nc.declare_dram_parameter("bqk", [128, 4], F32, isOutput=False)
    # out[p, ch, d, t] -> full[d*128+p, ch*512+t]
    out = nc.declare_dram_parameter("out", [128, QC * CT * TQ], BF16, isOutput=True)

    with tile.TileContext(nc) as tc:
        with tc.tile_pool(name="sb", bufs=1) as sb:
            # ---- persistent SBUF ----
            q_sb = [sb.tile([128, T], BF16, tag=f"q{p}", name=f"q{p}") for p in range(PAIRS)]
            k_sb = [sb.tile([128, T], BF16, tag=f"k{p}", name=f"k{p}") for p in range(PAIRS)]
            v_sb = [sb.tile([128, HC * VW], BF16, tag=f"v{t}", name=f"v{t}") for t in range(KT)]
            yn_sb = [sb.tile([128, T], BF16, tag=f"yn{p}", name=f"yn{p}") for p in range(PAIRS)]
            wpt_all = sb.tile([128, PAIRS, C], BF16, tag="wpt", name="wpt")
            bqk_sb = sb.tile([128, 4], F32, tag="bqk", name="bqk")
            ones_sb = sb.tile([1, HD], F32, tag="ones", name="ones")

            xtc = [sb.tile([128, CT, TQ], BF16, tag=f"xc{ch}", name=f"xc{ch}")
                   for ch in range(QC)]
            wtq = sb.tile([128, CT, 256], BF16, tag="wtq", name="wtq")
            wtk = sb.tile([128, CT, 256], BF16, tag="wtk", name="wtk")
            wtv = sb.tile([128, CT, 256], BF16, tag="wtv", name="wtv")

            nc.vector.memset(ones_sb[:, :], 1.0)
            for t in range(KT):
                vh = v_sb[t][:, :].rearrange("p (h c) -> p h c", c=VW)
                nc.vector.memset(vh[:, :, HD:HD + 1], 1.0)

            # batched contiguous DMAs, in need order
            def r3(ap, inner):
                return ap[:, :].rearrange("p (k c) -> p k c", c=inner)

            nc.scalar.dma_start(out=wtk[:, :, :], in_=r3(wk_d, 256))
            nc.scalar.dma_start(out=wtv[:, :, :], in_=r3(wv_d, 256))
            nc.scalar.dma_start(out=wtq[:, :, :], in_=r3(wq_d, 256))
            nc.scalar.dma_start(out=bqk_sb[:, :], in_=bqk[:, :])
            for ch in range(QC):
                nc.sync.dma_start(out=xtc[ch][:, :, :], in_=r3(xc_d[ch], TQ))
            nc.scalar.dma_start(out=wpt_all[:, :, :], in_=r3(wp_d, C))

            with tc.tile_pool(name="ps", bufs=1, space="PSUM") as ps:
                done = set()

                def emit_q(p, ch):
                    acc = ps.tile([128, TQ], F32, tag="lz", name="qacc", bufs=2)
                    for k in range(CT):
                        nc.tensor.matmul(
                            acc[:, :],
                            lhsT=wtq[:, k, 128 * p:128 * (p + 1)],
                            rhs=xtc[ch][:, k, :],
                            start=(k == 0), stop=(k == CT - 1),
                        )
                        yield None
                    nc.vector.tensor_scalar_add(
                        q_sb[p][:, TQ * ch:TQ * (ch + 1)], acc[:, :],
                        bqk_sb[:, p:p + 1],
                    )
                    done.add(("q", p, ch))

                def emit_k(p, ch):
                    acc = ps.tile([128, TQ], F32, tag="lz", name="kacc", bufs=2)
                    for k in range(CT):
                        nc.tensor.matmul(
                            acc[:, :],
                            lhsT=wtk[:, k, 128 * p:128 * (p + 1)],
                            rhs=xtc[ch][:, k, :],
                            start=(k == 0), stop=(k == CT - 1),
                        )
                        yield None
                    nc.vector.tensor_scalar_add(
                        k_sb[p][:, TQ * ch:TQ * (ch + 1)], acc[:, :],
                        bqk_sb[:, 2 + p:3 + p],
                    )
                    done.add(("k", p, ch))

                def emit_v(t):
                    # both pairs at once: N=256 amortizes per-matmul overhead
                    acc = ps.tile([128, 256], F32, tag="lz", name="vacc", bufs=2)
                    ch, tt = t // 4, t % 4
                    for k in range(CT):
                        nc.tensor.matmul(
                            acc[:, :],
                            lhsT=xtc[ch][:, k, 128 * tt:128 * (tt + 1)],
                            rhs=wtv[:, k, :],
                            start=(k == 0), stop=(k == CT - 1),
                        )
                        yield None
                    vh = v_sb[t][:, :].rearrange("p (h c) -> p h c", c=VW)
                    nc.vector.tensor_copy(
                        vh[:, :, 0:HD],
                        acc[:, :].rearrange("p (h c) -> p h c", c=HD),
                    )
                    done.add(("v", t))

                # ---- eager: chunk-0 work only; the rest is demand-pulled ----
                for gen in (emit_k(0, 0), emit_q(0, 0),
                            emit_v(0), emit_v(1), emit_v(2), emit_v(3)):
                    for _ in gen:
                        pass

                def lazy_stream():
                    for ch in range(1, QC):
                        yield from emit_k(0, ch)
                        for t in range(4 * ch, 4 * ch + 4):
                            yield from emit_v(t)
                        yield from emit_q(0, ch)
                    for ch in range(QC):
                        yield from emit_k(1, ch)
                    for ch in range(QC):
                        yield from emit_q(1, ch)

                lz = lazy_stream()

                def pump(n):
                    for _ in range(n):
                        next(lz, None)

                def require(key):
                    while key not in done:
                        if next(lz, "END") == "END":
                            assert key in done, f"lazy stream ended before {key}"

                # ---- attention + split deferred normalize + streaming proj ----
                sb_att = tc.alloc_tile_pool(name="sb_att", bufs=1)
                deferred = [None]   # (p, ch, yst, den, rc)
                proj_ready = []
                proj_queue = []

                def emit_recip(item):
                    p, ch, yst, den = item
                    rc = sb_att.tile([1, 2 * TQ], F32, tag="recip", name="recip", bufs=2)
                    nc.vector.reciprocal_approx_fast(rc[:, :], den[:, :])
                    return (p, ch, yst, den, rc)

                def emit_bc_mults(item):
                    # broadcast 1/denom across partitions on the idle GpSimd
                    # engine (SBUF->SBUF) instead of a PE ones-matmul: keeps
                    # ~16K cycles of matmul + PSUM ring churn out of the
                    # exp-paced attention window.
                    p, ch, yst, den, rc = item
                    bc = sb_att.tile([HD, 2 * TQ], F32, tag="bc", name="bc", bufs=2)
                    nc.gpsimd.partition_broadcast(bc[:, :], rc[:, :], channels=HD)
                    nc.vector.tensor_mul(
                        yn_sb[p][0:HD, TQ * ch:TQ * (ch + 1)],
                        yst[:, 0:TQ], bc[:, 0:TQ],
                    )
                    nc.vector.tensor_mul(
                        yn_sb[p][HD:128, TQ * ch:TQ * (ch + 1)],
                        yst[:, TQ:2 * TQ], bc[:, TQ:2 * TQ],
                    )
                    if p == PAIRS - 1:
                        proj_ready.append(ch)

                out_r = out[:, :].rearrange(
                    "p (c d t) -> p c d t", c=QC, d=CT
                )

                def emit_proj_chunk(ch):
                    ot = sb_att.tile([128, CT, TQ], BF16, tag="otmp", name="otmp", bufs=2)
                    eng = nc.scalar if ch == QC - 1 else nc.sync
                    for d in range(CT):
                        pacc = ps.tile([128, TQ], F32, tag="lz", name="pacc", bufs=2)
                        for p in range(PAIRS):
                            nc.tensor.matmul(
                                pacc[:, :],
                                lhsT=wpt_all[:, p, 128 * d:128 * (d + 1)],
                                rhs=yn_sb[p][:, TQ * ch:TQ * (ch + 1)],
                                start=(p == 0), stop=(p == PAIRS - 1),
                            )
                        nc.vector.tensor_copy(ot[:, d, :], pacc[:, :])
                        if d == CT // 2 - 1:
                            eng.dma_start(
                                out=out_r[:, ch, 0:CT // 2, :],
                                in_=ot[:, 0:CT // 2, :],
                            )
                        yield None
                    eng.dma_start(
                        out=out_r[:, ch, CT // 2:CT, :],
                        in_=ot[:, CT // 2:CT, :],
                    )

                def pump_proj(n):
                    for _ in range(n):
                        if proj_queue:
                            if next(proj_queue[0], "END") == "END":
                                proj_queue.pop(0)

                for p in range(PAIRS):
                    for ch in range(QC):
                        require(("q", p, ch))
                        if p > 0:
                            for cc in range(QC):
                                require(("k", p, cc))
                        yab = ps.tile([VW, 2 * TQ], F32, tag="yab", name="yab")
                        prev = [None]

                        def emit_pv(tt, pab):
                            if p == 0 and ch == 0:
                                require(("v", tt))
                            vh = v_sb[tt][:, :].rearrange("p (h c) -> p h c", c=VW)
                            nc.tensor.matmul(
                                yab[:, 0:TQ],
                                lhsT=vh[:, 2 * p, :],
                                rhs=pab[:, 0:TQ],
                                start=(tt == 0), stop=(tt == KT - 1),
                            )
                            nc.tensor.matmul(
                                yab[:, TQ:2 * TQ],
                                lhsT=vh[:, 2 * p + 1, :],
                                rhs=pab[:, TQ:2 * TQ],
                                start=(tt == 0), stop=(tt == KT - 1),
                            )

                        for t in range(KT):
                            if p == 0 and ch == 0 and t % 4 == 0 and t > 0:
                                require(("k", 0, t // 4))
                            sp = ps.tile([128, 2 * TQ], F32, tag="sp", name="sp", bufs=2)
                            nc.tensor.matmul(
                                sp[:, 0:TQ],
                                lhsT=k_sb[p][0:64, 128 * t:128 * (t + 1)],
                                rhs=q_sb[p][0:64, TQ * ch:TQ * (ch + 1)],
                                start=True, stop=True,
                            )
                            nc.tensor.matmul(
                                sp[:, TQ:2 * TQ],
                                lhsT=k_sb[p][64:128, 128 * t:128 * (t + 1)],
                                rhs=q_sb[p][64:128, TQ * ch:TQ * (ch + 1)],
                                start=True, stop=True,
                                tile_position=(64, 0),
                            )
                            pab = sb_att.tile([128, 2 * TQ], BF16, tag="pab", name="pab", bufs=4)
                            nc.scalar.activation(
                                pab[:, :], sp[:, :], AF.Exp, scale=float(SCALE),
                            )
                            if prev[0] is not None:
                                emit_pv(*prev[0])
                            prev[0] = (t, pab)
                            pump(6 if (p == 0 and ch == 0) else 3 if p == 0 else 2)
                            if t % 2 == 1:
                                pump_proj(1)
                            if t == 1 and deferred[0] is not None:
                                deferred[0] = emit_recip(deferred[0])
                            if t == 6 and deferred[0] is not None:
                                emit_bc_mults(deferred[0])
                                deferred[0] = None
                                while proj_ready:
                                    proj_queue.append(emit_proj_chunk(proj_ready.pop(0)))
                        emit_pv(*prev[0])
                        yst = sb_att.tile([HD, 2 * TQ], F32, tag="yst", name="yst", bufs=2)
                        den = sb_att.tile([1, 2 * TQ], F32, tag="den", name="den", bufs=2)
                        nc.vector.tensor_copy(yst[:, :], yab[0:HD, :])
                        nc.vector.tensor_copy(den[0:1, :], yab[HD:VW, :])
                        deferred[0] = (p, ch, yst, den)

                # ---- tail: last normalize + remaining proj ----
                pump(10 ** 9)
                emit_bc_mults(emit_recip(deferred[0]))
                while proj_ready:
                    proj_queue.append(emit_proj_chunk(proj_ready.pop(0)))
                pump_proj(10 ** 9)

                sb_att.release()

    nc.compile()
    return nc


def _get_nc():
    if "nc" not in _CACHE:
        _CACHE["nc"] = build_nc()
    return _CACHE["nc"]


def _to_tiles(a, nblk, blk):
    """[nblk*blk, F] -> [blk, nblk*F] with row p holding all nblk blocks."""
    F = a.shape[1]
    return np.ascontiguousarray(
        a.reshape(nblk, blk, F).transpose(1, 0, 2).reshape(blk, nblk * F)
    )


def make_in_maps(x, W_attn, b_attn, W_proj, b_proj):
    x = np.asarray(x, dtype=np.float32)
    W_attn = np.asarray(W_attn, dtype=np.float32)
    b_attn = np.asarray(b_attn, dtype=np.float32)
    W_proj = np.asarray(W_proj, dtype=np.float32)

    bf = ml_dtypes.bfloat16
    xTg = [np.ascontiguousarray(x[g].T).astype(bf) for g in range(B)]  # [C, T]

    in_maps = []
    for c in range(N_CORES):
        g, hq = divmod(c, G)
        r0 = hq * 256  # first row of this core's q slice within W_attn[0:C]
        wq = _to_tiles(W_attn[r0:r0 + 256].T.astype(bf), CT, 128)
        wk = _to_tiles(W_attn[C + r0:C + r0 + 256].T.astype(bf), CT, 128)
        wv = _to_tiles(W_attn[2 * C + r0:2 * C + r0 + 256].T.astype(bf), CT, 128)
        wp = _to_tiles(W_proj[:, r0:r0 + 256].T.astype(bf), PAIRS, 128)
        bq = b_attn[r0:r0 + 256].reshape(2, 128).T
        bk = b_attn[C + r0:C + r0 + 256].reshape(2, 128).T
        bqk_h = np.ascontiguousarray(
            np.concatenate([bq, bk], axis=1), dtype=np.float32
        )
        m = {"wq": wq, "wk": wk, "wv": wv, "wp": wp, "bqk": bqk_h}
        for ch in range(QC):
            m[f"xc{ch}"] = _to_tiles(xTg[g][:, TQ * ch:TQ * (ch + 1)], CT, 128)
        in_maps.append(m)
    return in_maps


def run_shards(in_maps, trace=False, **kw):
    nc = _get_nc()
    return run_bass_kernel_spmd(
        nc, in_maps, core_ids=list(range(N_CORES)), trace=trace, **kw
    )


def combine_outputs(res, W_proj, b_proj, b_attn):
    """Sum the 4 head-quad partial outputs per batch, add folded bias."""
    W_proj = np.asarray(W_proj, dtype=np.float32)
    b_proj = np.asarray(b_proj, dtype=np.float32)
    b_attn = np.asarray(b_attn, dtype=np.float32)
    b_adj = b_proj + W_proj @ b_attn[2 * C:]
    out = np.empty((B, T, C), dtype=np.float32)
    for g in range(B):
        acc = None
        for hq in range(G):
            r = res.results[g * G + hq]["out"].astype(np.float32)
            # r[p, ch, d, t] -> [d*128+p, ch*512+t]
            r = r.reshape(128, QC, CT, TQ).transpose(2, 0, 1, 3).reshape(C, T)
            acc = r if acc is None else acc + r
        out[g] = acc.T + b_adj
    return out


def kernel(x, W_attn, b_attn, W_proj, b_proj):
    in_maps = make_in_maps(x, W_attn, b_attn, W_proj, b_proj)
    res = run_shards(in_maps)
    return combine_outputs(res, W_proj, b_proj, b_attn)
